# revision 1
# baseline (speedup 1.0000x reference)
"""Trainium2 Bass kernel for nn_Block_39513699123558 (gnn_message_passing).

Two layers of (Chebyshev graph conv K=5 -> BatchNorm -> ReLU) on
x[B=2, F0=16, V=162, X=Y=Z=16].

Strategy (8 NeuronCores, data-parallel over B x S-quarters):
  - each core owns shard [Fin, V, S=1024] (b = core//4, s-quarter = core%4)
  - Chebyshev: host-precomputed stacked T_k matrices; one matmul family
    contracting V (lhsT = T-stack tiles, rhs = activations [v, (f s)])
  - layout bridge xs[(k u), (f s)] -> xsT[(k f), (u s)] via per-row SBUF DMAs
  - projection: matmul contracting (k f) -> y[o, (u s)].  The vertex axis is
    split into four bands (42/42/42/36); band j computes on PE column strip j
    (tile_position=(0,32j)) so four matmuls share one [128, 384] PSUM tile.
    y-slab rows = 32*j + o, free = (u-in-band slots, s: 1024).
  - BN: bn_stats straight off each shared PSUM tile (contiguous, 6/partition),
    bn_aggr at layer end, per-row count-weighted (E, S=var+mean^2) AllReduce
    across cores, band-fold on 32 rows, then per-partition scale/shift + ReLU.
    The conv bias cancels exactly inside BatchNorm and is dropped.
All matmul data bf16; PSUM/stats/normalization math f32; output f32.
"""

import os
import sys

sys.path.insert(0, "/opt/trn_rl_repo")

SKIP_CC = os.environ.get("K_SKIP_CC", "0") == "1"
CC_MODE = os.environ.get("K_CC", "both")  # none|l1|l2|both


import numpy as np
import ml_dtypes

from concourse import bass, bacc, mybir
from concourse import tile
from concourse.bass_utils import run_bass_kernel_spmd

BF16 = ml_dtypes.bfloat16
BF = mybir.dt.bfloat16
F32 = mybir.dt.float32

V = 162
VA = 128
VB = V - VA  # 34
F1, F2 = 16, 32
K = 5
S = 1024          # s-columns per core
SC = 64           # s-chunk (both layers)
NCH = S // SC     # 16
EPS = 1e-5
N_CORES = 8

# vertex bands per PE column strip: u in [UB[j], UB[j+1])
UB = [0, 42, 84, 126, 162]
BW = [42, 42, 42, 36]
NG = 7            # 6-u-row slots per band (band 3 uses 6 of 7)

# L1 stack: k=1..4 (T_0 = I handled from x directly), 648 rows, 6 tiles of 108
ST1 = [108] * 6
SP1 = [(1, 0, 0, 0, 108), (1, 1, 0, 108, 54),
       (2, 1, 54, 0, 54), (2, 2, 0, 54, 108),
       (3, 3, 0, 0, 108), (3, 4, 0, 108, 54),
       (4, 4, 54, 0, 54), (4, 5, 0, 54, 108)]
# L2 stack: k=0..4 (T_0 = I included), 810 rows, tiles [128]*6 + [42]
ST2 = [128] * 6 + [42]


def _spans(tiles, nk):
    bounds = np.cumsum([0] + tiles)
    out = []
    for k in range(nk):
        r = 162 * k
        while r < 162 * (k + 1):
            t = int(np.searchsorted(bounds, r, side="right") - 1)
            span = int(min(bounds[t + 1], 162 * (k + 1)) - r)
            out.append((k, t, int(r - bounds[t]), r - 162 * k, span))
            r += span
    return out


SP2 = _spans(ST2, K)

NSUB = 384                    # m2 column sub: 6 u-rows x 64 s
# bn stats scratch: (chunk, g) slots of 8 (6 stats + 2 pad)
NSLOT = NCH * NG              # 112
STSCR_W = NSLOT * 8


def build_program():
    nc = bacc.Bacc("TRN2", target_bir_lowering=False)
    xk = nc.declare_dram_parameter("xk", [V, NCH, F1, SC], BF, False)
    tsk1 = nc.declare_dram_parameter("tsk1", [V, 648], BF, False)
    tsk2 = nc.declare_dram_parameter("tsk2", [V, 810], BF, False)
    w1r = nc.declare_dram_parameter("w1r", [K * F1, F2], BF, False)
    w2a = nc.declare_dram_parameter("w2a", [128, F2], BF, False)
    w2b = nc.declare_dram_parameter("w2b", [32, F2], BF, False)
    gb1 = nc.declare_dram_parameter("gb1", [128, 2], F32, False)
    gb2 = nc.declare_dram_parameter("gb2", [128, 2], F32, False)
    wrow = nc.declare_dram_parameter("wrow", [128, 1], F32, False)
    out = nc.declare_dram_parameter("out", [F2, V, S], F32, isOutput=True)

    with tile.TileContext(nc) as tc:
        with (
            tc.tile_pool(name="consts", bufs=1) as cpool,
            tc.tile_pool(name="slab", bufs=1) as slab,
            tc.tile_pool(name="stats", bufs=1) as spool,
            tc.tile_pool(name="dram", bufs=1, space="DRAM") as dram,
        ):
            tA1 = cpool.tile([VA, 648], BF)
            tB1 = cpool.tile([VB, 648], BF)
            tA2 = cpool.tile([VA, 810], BF)
            tB2 = cpool.tile([VB, 810], BF)
            w1t = cpool.tile([K * F1, F2], BF)
            w2at = cpool.tile([128, F2], BF)
            w2bt = cpool.tile([32, F2], BF)
            gb1t = cpool.tile([128, 2], F32)
            gb2t = cpool.tile([128, 2], F32)
            wrt = cpool.tile([128, 1], F32)
            nc.sync.dma_start(tA1[:], tsk1[0:VA, :])
            nc.sync.dma_start(tB1[:], tsk1[VA:V, :])
            nc.sync.dma_start(tA2[:], tsk2[0:VA, :])
            nc.sync.dma_start(tB2[:], tsk2[VA:V, :])
            nc.sync.dma_start(w1t[:], w1r[:])
            nc.sync.dma_start(w2at[:], w2a[:])
            nc.sync.dma_start(w2bt[:], w2b[:])
            nc.sync.dma_start(gb1t[:], gb1[:])
            nc.sync.dma_start(gb2t[:], gb2[:])
            nc.sync.dma_start(wrt[:], wrow[:])

            # y-slab rows 32j+o; free = (g: 7 u-slots, r: 6, s: 1024)
            yslab = slab.tile([128, NG * 6 * S], BF)
            ysl = yslab[:, :].rearrange("p (g r s) -> p g r s",
                                        g=NG, r=6, s=S)
            stscr1 = spool.tile([128, STSCR_W], F32)
            stscr2 = spool.tile([128, STSCR_W], F32)
            par1 = spool.tile([128, 2], F32)
            par2 = spool.tile([128, 2], F32)
            nc.gpsimd.memset(stscr1[:], 0.0)
            nc.gpsimd.memset(stscr2[:], 0.0)

            def proj_evac(c, lhs_list, xsT_list, stscr, mxps):
                # projection for chunk c: band j on column strip j; the four
                # bands share one [128, NSUB] psum tile per u-slot g
                sl = c * SC
                for g in range(NG):
                    ps2 = mxps.tile([128, NSUB], F32, tag="m2ps")
                    rows = 128 if g < 6 else 96
                    for j in range(4):
                        if g == 6 and j == 3:
                            continue
                        col0 = (UB[j] + 6 * g) * SC
                        for i, (lw, xsT) in enumerate(zip(lhs_list, xsT_list)):
                            nc.tensor.matmul(
                                ps2[32 * j:32 * j + 32, :], lw[:],
                                xsT[:, col0:col0 + NSUB],
                                start=(i == 0), stop=(i == len(lhs_list) - 1),
                                tile_position=(0, 32 * j))
                    slot = c * NG + g
                    st = stscr[0:rows, slot * 8:slot * 8 + 6]
                    nc.vector.bn_stats(st, ps2[0:rows, :])
                    dst = ysl[0:rows, g, :, sl:sl + SC]
                    src = ps2[0:rows, :].rearrange("p (r s) -> p r s",
                                                   r=6, s=SC)
                    if g % 2 == 0:
                        nc.vector.tensor_copy(dst, src)
                    else:
                        nc.scalar.copy(dst, src)

            def layer1(c, xpool, m1ps, m2ps, xsp, xtp):
                xa = xpool.tile([VA, F1 * SC], BF, tag="xa")
                xb = xpool.tile([VB, F1 * SC], BF, tag="xb")
                nc.sync.dma_start(xa[:], xk[0:VA, c, :, :])
                nc.sync.dma_start(xb[:], xk[VA:V, c, :, :])
                xs = []
                for m in range(6):
                    ps0 = m1ps.tile([108, 512], F32, tag="m1ps")
                    ps1 = m1ps.tile([108, 512], F32, tag="m1ps")
                    for kc, (tt, xx) in enumerate(((tA1, xa), (tB1, xb))):
                        lw = tt[:, m * 108:(m + 1) * 108]
                        st = dict(start=(kc == 0), stop=(kc == 1))
                        nc.tensor.matmul(ps0[:], lw, xx[:, 0:512], **st)
                        nc.tensor.matmul(ps1[:], lw, xx[:, 512:1024], **st)
                    xt = xsp.tile([108, F1 * SC], BF, tag=f"xs{m}")
                    if m % 2 == 0:
                        nc.vector.tensor_copy(xt[:, 0:512], ps0[:])
                        nc.vector.tensor_copy(xt[:, 512:1024], ps1[:])
                    else:
                        nc.scalar.copy(xt[:, 0:512], ps0[:])
                        nc.scalar.copy(xt[:, 512:1024], ps1[:])
                    xs.append(xt)
                xsT = xtp.tile([K * F1, V * SC], BF, tag="xsT")
                for f in range(F1):
                    nc.sync.dma_start(
                        xsT[f:f + 1, 0:VA * SC], xa[:, f * SC:(f + 1) * SC])
                    nc.sync.dma_start(
                        xsT[f:f + 1, VA * SC:V * SC],
                        xb[:, f * SC:(f + 1) * SC])
                for (k, t, r0, u0, span) in SP1:
                    for f in range(F1):
                        r = k * F1 + f
                        nc.sync.dma_start(
                            xsT[r:r + 1, u0 * SC:(u0 + span) * SC],
                            xs[t][r0:r0 + span, f * SC:(f + 1) * SC])
                proj_evac(c, [w1t], [xsT], stscr1, m2ps)

            def layer2(c, h1p, m1ps, m2ps, xsp, xtp, xtpb):
                sl = c * SC
                # JIT normalize+relu of this s-slice (all bands), in place
                nc.scalar.activation(
                    ysl[:, :, :, sl:sl + SC], ysl[:, :, :, sl:sl + SC],
                    mybir.ActivationFunctionType.Relu,
                    bias=par1[:, 1:2], scale=par1[:, 0:1])
                # bridge 2a: y-slab rows -> h1T [v, (f2 s)]
                ha = h1p.tile([VA, F2 * SC], BF, tag="ha")
                hb = h1p.tile([VB, F2 * SC], BF, tag="hb")
                for o in range(F2):
                    for j in range(4):
                        r = 32 * j + o
                        src = ysl[r:r + 1, :, :, sl:sl + SC]
                        u0, u1 = UB[j], UB[j + 1]
                        if u1 <= VA:
                            nc.sync.dma_start(
                                ha[u0:u1, o * SC:(o + 1) * SC],
                                src[:, 0:7, :, :])
                        else:
                            # band 3 (u 126..161) splits across ha/hb
                            nc.sync.dma_start(
                                ha[126:128, o * SC:(o + 1) * SC],
                                src[:, 0, 0:2, :])
                            nc.sync.dma_start(
                                hb[0:4, o * SC:(o + 1) * SC],
                                src[:, 0, 2:6, :])
                            nc.sync.dma_start(
                                hb[4:34, o * SC:(o + 1) * SC],
                                src[:, 1:6, :, :])
                xs = []
                off = 0
                for m, rows in enumerate(ST2):
                    pss = []
                    for _j in range(4):
                        psj = m1ps.tile([rows, 512], F32, tag="m1ps",
                                        name=f"ps2_{m}_{_j}")
                        pss.append(psj)
                    for kc, (tt, hh) in enumerate(((tA2, ha), (tB2, hb))):
                        lw = tt[:, off:off + rows]
                        st = dict(start=(kc == 0), stop=(kc == 1))
                        for j in range(4):
                            nc.tensor.matmul(
                                pss[j][:], lw, hh[:, j * 512:(j + 1) * 512],
                                **st)
                    xt = xsp.tile([rows, F2 * SC], BF, tag=f"x2_{m}")
                    for j in range(4):
                        if (m + j) % 2 == 0:
                            nc.vector.tensor_copy(
                                xt[:, j * 512:(j + 1) * 512], pss[j][:])
                        else:
                            nc.scalar.copy(
                                xt[:, j * 512:(j + 1) * 512], pss[j][:])
                    xs.append(xt)
                    off += rows
                xsTa = xtp.tile([128, V * SC], BF, tag="xsTa")
                xsTb = xtpb.tile([32, V * SC], BF, tag="xsTb")
                for (k, t, r0, u0, span) in SP2:
                    for f in range(F2):
                        dst = (xsTa[k * F2 + f:k * F2 + f + 1]
                               if k < 4 else xsTb[f:f + 1])
                        nc.sync.dma_start(
                            dst[:, u0 * SC:(u0 + span) * SC],
                            xs[t][r0:r0 + span, f * SC:(f + 1) * SC])
                proj_evac(c, [w2at, w2bt], [xsTa, xsTb], stscr2, m2ps)

            def bn_finalize(stscr, gbt, par, tag):
                # per-row (mean, var) -> count-weighted (E, S) -> AllReduce ->
                # band-fold -> scale/shift
                sv = stscr[:, :].rearrange("p (n e) -> p n e", n=NSLOT, e=8)
                mv = spool.tile([128, 2], F32, tag=f"mv{tag}")
                nc.vector.bn_aggr(mv[:], sv[:, :, 0:6])
                es = spool.tile([128, 2], F32, tag=f"es{tag}")
                nc.vector.tensor_mul(es[:, 1:2], mv[:, 0:1], mv[:, 0:1])
                nc.vector.tensor_add(es[:, 1:2], es[:, 1:2], mv[:, 1:2])
                nc.vector.tensor_copy(es[:, 0:1], mv[:, 0:1])
                nc.vector.tensor_mul(es[:, 0:1], es[:, 0:1], wrt[:, 0:1])
                nc.vector.tensor_mul(es[:, 1:2], es[:, 1:2], wrt[:, 0:1])
                cin = dram.tile([128, 2], F32, tag=f"cin{tag}")
                cout = dram.tile([128, 2], F32, tag=f"cout{tag}")
                nc.gpsimd.dma_start(cin[:], es[:])
                use_cc = (CC_MODE == "both" or CC_MODE == ("l" + tag)) and not SKIP_CC
                if use_cc:
                    nc.gpsimd.collective_compute(
                        "AllReduce", mybir.AluOpType.add,
                        replica_groups=[list(range(N_CORES))],
                        ins=[cin[:].opt()], outs=[cout[:].opt()])
                else:
                    nc.gpsimd.dma_start(cout[:], cin[:])
                qs = spool.tile([32, 8], F32, tag=f"qs{tag}")
                nc.sync.dma_start(
                    qs[:].rearrange("o (j e) -> o j e", j=4, e=2),
                    cout[:].rearrange("(j o) e -> o j e", j=4, o=32))
                acc = spool.tile([32, 6], F32, tag=f"acc{tag}")
                nc.vector.tensor_add(acc[:, 0:2], qs[:, 0:2], qs[:, 2:4])
                nc.vector.tensor_add(acc[:, 2:4], qs[:, 4:6], qs[:, 6:8])
                nc.vector.tensor_add(acc[:, 0:2], acc[:, 0:2], acc[:, 2:4])
                # acc[:,0]=global mean, acc[:,1]=global E[y^2]
                nc.vector.tensor_mul(acc[:, 2:3], acc[:, 0:1], acc[:, 0:1])
                nc.vector.tensor_sub(acc[:, 1:2], acc[:, 1:2], acc[:, 2:3])
                nc.vector.tensor_scalar_add(acc[:, 1:2], acc[:, 1:2], EPS)
                nc.scalar.sqrt(acc[:, 2:3], acc[:, 1:2])
                nc.vector.reciprocal(acc[:, 3:4], acc[:, 2:3])
                nc.vector.tensor_mul(acc[:, 4:5], gbt[0:32, 0:1], acc[:, 3:4])
                nc.vector.tensor_mul(acc[:, 5:6], acc[:, 0:1], acc[:, 4:5])
                nc.vector.tensor_sub(acc[:, 5:6], gbt[0:32, 1:2], acc[:, 5:6])
                for j in range(4):
                    nc.sync.dma_start(par[32 * j:32 * j + 32, 0:1],
                                      acc[:, 4:5])
                    nc.sync.dma_start(par[32 * j:32 * j + 32, 1:2],
                                      acc[:, 5:6])

            # ---- layer 1 ----
            with (
                tc.tile_pool(name="x", bufs=3) as xpool,
                tc.tile_pool(name="m1ps", bufs=4, space="PSUM") as m1ps,
                tc.tile_pool(name="m2ps", bufs=3, space="PSUM") as m2ps,
                tc.tile_pool(name="xs", bufs=2) as xsp,
                tc.tile_pool(name="xsT", bufs=2) as xtp,
            ):
                for c in range(NCH):
                    layer1(c, xpool, m1ps, m2ps, xsp, xtp)
            bn_finalize(stscr1, gb1t, par1, "1")

            # ---- layer 2 ----
            with (
                tc.tile_pool(name="h1", bufs=2) as h1p,
                tc.tile_pool(name="m1ps2", bufs=4, space="PSUM") as m1ps,
                tc.tile_pool(name="m2ps2", bufs=3, space="PSUM") as m2ps,
                tc.tile_pool(name="xs2", bufs=1) as xsp,
                tc.tile_pool(name="xsT2a", bufs=2) as xtp,
                tc.tile_pool(name="xsT2b", bufs=1) as xtpb,
            ):
                for c in range(NCH):
                    layer2(c, h1p, m1ps, m2ps, xsp, xtp, xtpb)
            bn_finalize(stscr2, gb2t, par2, "2")

            # ---- final normalize + relu + store ----
            with tc.tile_pool(name="stg", bufs=3) as stg:
                for c in range(NCH):
                    sl = c * SC
                    so = stg.tile([128, NG * 6 * SC], F32, tag="stg")
                    so4 = so[:, :].rearrange("p (g r s) -> p g r s",
                                             g=NG, r=6, s=SC)
                    nc.scalar.activation(
                        so4, ysl[:, :, :, sl:sl + SC],
                        mybir.ActivationFunctionType.Relu,
                        bias=par2[:, 1:2], scale=par2[:, 0:1])
                    for j in range(4):
                        u0, u1 = UB[j], UB[j + 1]
                        srcv = so[32 * j:32 * j + 32,
                                  0:(u1 - u0) * SC].rearrange(
                            "p (u s) -> p u s", u=u1 - u0, s=SC)
                        nc.sync.dma_start(out[:, u0:u1, sl:sl + SC], srcv)
    nc.compile()
    return nc


def _host_prep(x, lap, w1, w2, g1, be1, g2, be2):
    lap64 = np.asarray(lap).astype(np.float64)
    T = [np.eye(V), lap64]
    for _ in range(2, K):
        T.append(2.0 * lap64 @ T[-1] - T[-2])
    tsk1 = np.concatenate([T[k].T for k in range(1, K)], axis=1)  # [162, 648]
    tsk2 = np.concatenate([T[k].T for k in range(0, K)], axis=1)  # [162, 810]
    w1r = np.asarray(w1).reshape(K * F1, F2)
    w2r = np.asarray(w2).reshape(K * F2, F2)
    gb1 = np.stack([np.tile(np.asarray(g1), 4), np.tile(np.asarray(be1), 4)],
                   axis=1)
    gb2 = np.stack([np.tile(np.asarray(g2), 4), np.tile(np.asarray(be2), 4)],
                   axis=1)
    # per-row weight: n_row / total; rows 32j+o weigh band j
    nrow = np.repeat(np.array(BW, np.float64) * S, 32)
    denom = (1.0 if os.environ.get("K_SKIP_CC", "0") == "1" else 8.0) * V * S
    wrow = (nrow / denom).astype(np.float32)[:, None]
    common = {
        "tsk1": tsk1.astype(BF16), "tsk2": tsk2.astype(BF16),
        "w1r": w1r.astype(BF16),
        "w2a": w2r[:128].astype(BF16), "w2b": w2r[128:].astype(BF16),
        "gb1": gb1.astype(np.float32), "gb2": gb2.astype(np.float32),
        "wrow": wrow,
    }
    in_maps = []
    xf = np.asarray(x).reshape(2, F1, V, 4096)
    for core in range(N_CORES):
        b, q = core // 4, core % 4
        xs = xf[b, :, :, q * S:(q + 1) * S]            # [16, 162, 1024]
        xs = xs.transpose(1, 0, 2).reshape(V, F1, NCH, SC)
        xs = xs.transpose(0, 2, 1, 3)                  # [162, 16, 16, 64]
        m = dict(common)
        m["xk"] = np.ascontiguousarray(xs).astype(BF16)
        in_maps.append(m)
    return in_maps


_CACHE = {}


def _run(in_maps, trace=False):
    if "nc" not in _CACHE:
        _CACHE["nc"] = build_program()
    return run_bass_kernel_spmd(
        _CACHE["nc"], in_maps, core_ids=list(range(N_CORES)), trace=trace)


def kernel(x, lap, w1, b1, g1, be1, w2, b2, g2, be2, _trace=False):
    # conv biases b1/b2 cancel exactly inside BatchNorm -> ignored
    in_maps = _host_prep(x, lap, w1, w2, g1, be1, g2, be2)
    res = _run(in_maps, trace=_trace)
    _CACHE["last_results"] = res
    full = np.empty((2, F2, V, 4096), np.float32)
    for core in range(N_CORES):
        b, q = core // 4, core % 4
        full[b, :, :, q * S:(q + 1) * S] = res.results[core]["out"]
    return full.reshape(2, F2, V, 16, 16, 16)



# revision 6
# speedup vs baseline: 5.7541x; 5.7541x over previous
"""Trainium2 Bass kernel for nn_Block_39513699123558 (gnn_message_passing).

Two layers of (Chebyshev graph conv K=5 -> BatchNorm -> ReLU) on
x[B=2, F0=16, V=162, X=Y=Z=16].

Strategy (8 NeuronCores, data-parallel over B x S-quarters):
  - each core owns shard [Fin, V, S=1024] (b = core//4, s-quarter = core%4)
  - s is processed in NCH=8 chunks of SC=128 columns
  - Chebyshev k=1..4 via one host-precomputed T-stack (shared by both
    layers, T_k symmetric); T_0 = identity handled without matmuls
  - conv: matmul contracting V (lhsT = T-stack tiles, rhs = activations
    [v, (f s)]) -> xs[(k u), (f s)]
  - layout bridge xs -> xsT[(k f), (u s)] goes through small DRAM staging
    buffers: per-tile span writes (gpsimd software DGE, cheap descriptors)
    + one contiguous read back per chunk.  This keeps the dma_start count
    ~30x below the per-row SBUF bridge (each dma_start costs ~0.6us on
    the issuing sequencer + HWDGE).
  - L1 projection: single matmul contracting (k f)=80 (identity rows of
    xsT filled from a host-transposed copy of x read straight from HBM).
    Vertex axis split in 4 bands of 42/42/42/36 on PE column strips.
  - L2 projection: 2 accumulated matmuls: k=1..4 from xsT2 (128 rows)
    + k=0 read directly from the normalized y1 slab (rhs partitions
    32j..32j+32, replicated w2[0] weights, tile_position (32j, 32j)).
  - projection psum tiles cover r=3 vertex rows x SC per band strip;
    bn_stats off each psum tile, count-weighted AllReduce at layer end,
    per-partition scale/shift + ReLU applied lazily (JIT) next pass.
  - proj of chunk c is emitted after conv of chunk c+1 so the DRAM
    read-back latency hides under conv; xsT pools stay single/double
    buffered within SBUF limits.
All matmul data bf16; PSUM/stats/normalization math f32; output f32.
"""

import os
import sys

sys.path.insert(0, "/opt/trn_rl_repo")

SKIP_CC = os.environ.get("K_SKIP_CC", "0") == "1"
CC_MODE = os.environ.get("K_CC", "both")  # none|l1|l2|both


import numpy as np
import ml_dtypes

from concourse import bass, bacc, mybir
from concourse import tile
from concourse.bass_utils import run_bass_kernel_spmd

BF16 = ml_dtypes.bfloat16
BF = mybir.dt.bfloat16
F32 = mybir.dt.float32

V = 162
VA = 128
VB = V - VA  # 34
F1, F2 = 16, 32
K = 5
S = 1024          # s-columns per core
SC = 128          # s-chunk
NCH = S // SC     # 8
EPS = 1e-5
N_CORES = 8

# vertex bands per PE column strip: u in [UB[j], UB[j+1])
UB = [0, 42, 84, 126, 162]
BW = [42, 42, 42, 36]
R = 3             # u-rows per proj psum slot
NG = 14           # slots per band (band 3 uses 12 of 14)
NGJ = [14, 14, 14, 12]
CW = 42 * SC      # yslab cols per chunk

# T-stack k=1..4 (T_0 = I handled separately), 648 rows, 6 tiles of 108
ST = [108] * 6
SP = [(1, 0, 0, 0, 108), (1, 1, 0, 108, 54),
      (2, 1, 54, 0, 54), (2, 2, 0, 54, 108),
      (3, 3, 0, 0, 108), (3, 4, 0, 108, 54),
      (4, 4, 54, 0, 54), (4, 5, 0, 54, 108)]
SP_BY_T = [[sp for sp in SP if sp[1] == t] for t in range(6)]

NSUB = R * SC                 # proj psum cols (f32)
NSLOT = NCH * NG              # 112 bn-stats slots (chunk, g)
STSCR_W = NSLOT * 8


def build_program():
    nc = bacc.Bacc("TRN2", target_bir_lowering=False)
    xk = nc.declare_dram_parameter("xk", [V, NCH, F1, SC], BF, False)
    xkt = nc.declare_dram_parameter("xkt", [NCH, F1, V * SC], BF, False)
    tsk = nc.declare_dram_parameter("tsk", [V, 648], BF, False)
    w1r = nc.declare_dram_parameter("w1r", [K * F1, F2], BF, False)
    w2k14 = nc.declare_dram_parameter("w2k14", [128, F2], BF, False)
    w2k0q = nc.declare_dram_parameter("w2k0q", [128, F2], BF, False)
    gb1 = nc.declare_dram_parameter("gb1", [128, 2], F32, False)
    gb2 = nc.declare_dram_parameter("gb2", [128, 2], F32, False)
    wrow = nc.declare_dram_parameter("wrow", [128, 1], F32, False)
    out = nc.declare_dram_parameter("out", [F2, V, S], F32, isOutput=True)

    with tile.TileContext(nc) as tc:
        with (
            tc.tile_pool(name="consts", bufs=1) as cpool,
            tc.tile_pool(name="slab", bufs=1) as slab,
            tc.tile_pool(name="stats", bufs=1) as spool,
            tc.tile_pool(name="dram", bufs=1, space="DRAM") as dram,
        ):
            tA = cpool.tile([VA, 648], BF)
            tB = cpool.tile([VB, 648], BF)
            w1t = cpool.tile([K * F1, F2], BF)
            w2at = cpool.tile([128, F2], BF)
            w2bt = cpool.tile([128, F2], BF)
            gb1t = cpool.tile([128, 2], F32)
            gb2t = cpool.tile([128, 2], F32)
            wrt = cpool.tile([128, 1], F32)
            nc.sync.dma_start(tA[:], tsk[0:VA, :])
            nc.sync.dma_start(tB[:], tsk[VA:V, :])
            nc.sync.dma_start(w1t[:], w1r[:])
            nc.sync.dma_start(w2at[:], w2k14[:])
            nc.sync.dma_start(w2bt[:], w2k0q[:])
            nc.sync.dma_start(gb1t[:], gb1[:])
            nc.sync.dma_start(gb2t[:], gb2[:])
            nc.sync.dma_start(wrt[:], wrow[:])

            # y-slab: rows 32j+o; cols per chunk c: (u-local 42, s SC)
            yslab = slab.tile([128, NCH * CW], BF)

            def ysl_c(c):
                return yslab[:, c * CW:(c + 1) * CW].rearrange(
                    "p (u s) -> p u s", u=42, s=SC)

            stscr1 = spool.tile([128, STSCR_W], F32)
            stscr2 = spool.tile([128, STSCR_W], F32)
            par1 = spool.tile([128, 2], F32)
            par2 = spool.tile([128, 2], F32)
            nc.gpsimd.memset(stscr1[:], 0.0)
            nc.gpsimd.memset(stscr2[:], 0.0)

            def proj_evac(c, stscr, mxps, mm_fn):
                # projection for chunk c: band j on column strip j; the
                # bands share one [128, NSUB] psum tile per u-slot g.
                # mm_fn(ps2, j, col0, c) emits the matmul(s) for a strip.
                ysl = ysl_c(c)
                for g in range(NG):
                    rows = 128 if g < 12 else 96
                    nj = 4 if g < 12 else 3
                    ps2 = mxps.tile([128, NSUB], F32, tag="m2ps")
                    for j in range(nj):
                        col0 = (UB[j] + R * g) * SC
                        mm_fn(ps2, j, col0, c)
                    slot = c * NG + g
                    st = stscr[0:rows, slot * 8:slot * 8 + 6]
                    nc.vector.bn_stats(st, ps2[0:rows, :])
                    dst = ysl[0:rows, R * g:R * g + R, :]
                    src = ps2[0:rows, :].rearrange("p (r s) -> p r s",
                                                   r=R, s=SC)
                    if g % 2 == 0:
                        nc.vector.tensor_copy(dst, src)
                    else:
                        nc.scalar.copy(dst, src)

            def conv_spans(c, rhs_pair, nf, xsp, xsd, m1ps, eng_flip):
                # graph-conv matmuls (k=1..4) + psum evac + DRAM span
                # writes for chunk c.  rhs_pair = (xa, xb) tiles
                # [V-part, nf*SC]; writes xsd rows (k-1)*nf+f.
                nspl = nf * SC // 512
                for m in range(6):
                    xst = xsp.tile([108, nf * SC], BF, tag="xst")
                    for spl in range(nspl):
                        ps = m1ps.tile([108, 512], F32, tag="m1ps")
                        for kc, (tt, xx) in enumerate(
                                ((tA, rhs_pair[0]), (tB, rhs_pair[1]))):
                            nc.tensor.matmul(
                                ps[:], tt[:, m * 108:(m + 1) * 108],
                                xx[:, spl * 512:(spl + 1) * 512],
                                start=(kc == 0), stop=(kc == 1))
                        dst = xst[:, spl * 512:(spl + 1) * 512]
                        if (m + spl + eng_flip) % 2 == 0:
                            nc.vector.tensor_copy(dst, ps[:])
                        else:
                            nc.scalar.copy(dst, ps[:])
                    for (k, t, r0, u0, span) in SP_BY_T[m]:
                        nc.gpsimd.dma_start(
                            xsd[(k - 1) * nf:k * nf,
                                u0 * SC:(u0 + span) * SC].rearrange(
                                "f (u s) -> u f s", u=span, s=SC),
                            xst[r0:r0 + span, :].rearrange(
                                "u (f s) -> u f s", f=nf, s=SC))

            def mm_l1(xsT):
                def mm(ps2, j, col0, c):
                    nc.tensor.matmul(
                        ps2[32 * j:32 * j + 32, :], w1t[:],
                        xsT[:, col0:col0 + NSUB],
                        start=True, stop=True, tile_position=(0, 32 * j))
                return mm

            def mm_l2(xsT2):
                def mm(ps2, j, col0, c):
                    nc.tensor.matmul(
                        ps2[32 * j:32 * j + 32, :], w2at[:],
                        xsT2[:, col0:col0 + NSUB],
                        start=True, stop=False, tile_position=(0, 32 * j))
                    g = (col0 // SC - UB[j]) // R
                    nc.tensor.matmul(
                        ps2[32 * j:32 * j + 32, :],
                        w2bt[32 * j:32 * j + 32, :],
                        ysl_c(c)[32 * j:32 * j + 32,
                                 R * g:R * g + R, :],
                        start=False, stop=True,
                        tile_position=(32 * j, 32 * j))
                return mm

            def bn_finalize(stscr, gbt, par, tag):
                # per-row (mean, var) -> count-weighted (E, S) -> AllReduce
                # -> band-fold -> scale/shift
                sv = stscr[:, :].rearrange("p (n e) -> p n e", n=NSLOT, e=8)
                mv = spool.tile([128, 2], F32, tag=f"mv{tag}")
                nc.vector.bn_aggr(mv[:], sv[:, :, 0:6])
                es = spool.tile([128, 2], F32, tag=f"es{tag}")
                nc.vector.tensor_mul(es[:, 1:2], mv[:, 0:1], mv[:, 0:1])
                nc.vector.tensor_add(es[:, 1:2], es[:, 1:2], mv[:, 1:2])
                nc.vector.tensor_copy(es[:, 0:1], mv[:, 0:1])
                nc.vector.tensor_mul(es[:, 0:1], es[:, 0:1], wrt[:, 0:1])
                nc.vector.tensor_mul(es[:, 1:2], es[:, 1:2], wrt[:, 0:1])
                cin = dram.tile([128, 2], F32, tag=f"cin{tag}")
                cout = dram.tile([128, 2], F32, tag=f"cout{tag}")
                nc.gpsimd.dma_start(cin[:], es[:])
                use_cc = (CC_MODE == "both" or CC_MODE == ("l" + tag)) \
                    and not SKIP_CC
                if use_cc:
                    nc.gpsimd.collective_compute(
                        "AllReduce", mybir.AluOpType.add,
                        replica_groups=[list(range(N_CORES))],
                        ins=[cin[:].opt()], outs=[cout[:].opt()])
                else:
                    nc.gpsimd.dma_start(cout[:], cin[:])
                qs = spool.tile([32, 8], F32, tag=f"qs{tag}")
                nc.sync.dma_start(
                    qs[:].rearrange("o (j e) -> o j e", j=4, e=2),
                    cout[:].rearrange("(j o) e -> o j e", j=4, o=32))
                acc = spool.tile([32, 6], F32, tag=f"acc{tag}")
                nc.vector.tensor_add(acc[:, 0:2], qs[:, 0:2], qs[:, 2:4])
                nc.vector.tensor_add(acc[:, 2:4], qs[:, 4:6], qs[:, 6:8])
                nc.vector.tensor_add(acc[:, 0:2], acc[:, 0:2], acc[:, 2:4])
                # acc[:,0]=global mean, acc[:,1]=global E[y^2]
                nc.vector.tensor_mul(acc[:, 2:3], acc[:, 0:1], acc[:, 0:1])
                nc.vector.tensor_sub(acc[:, 1:2], acc[:, 1:2], acc[:, 2:3])
                nc.vector.tensor_scalar_add(acc[:, 1:2], acc[:, 1:2], EPS)
                nc.scalar.sqrt(acc[:, 2:3], acc[:, 1:2])
                nc.vector.reciprocal(acc[:, 3:4], acc[:, 2:3])
                nc.vector.tensor_mul(acc[:, 4:5], gbt[0:32, 0:1], acc[:, 3:4])
                nc.vector.tensor_mul(acc[:, 5:6], acc[:, 0:1], acc[:, 4:5])
                nc.vector.tensor_sub(acc[:, 5:6], gbt[0:32, 1:2], acc[:, 5:6])
                for j in range(4):
                    nc.sync.dma_start(par[32 * j:32 * j + 32, 0:1],
                                      acc[:, 4:5])
                    nc.sync.dma_start(par[32 * j:32 * j + 32, 1:2],
                                      acc[:, 5:6])

            # ---- layer 1 ----
            with (
                tc.tile_pool(name="x", bufs=2) as xpool,
                tc.tile_pool(name="m1ps", bufs=4, space="PSUM") as m1ps,
                tc.tile_pool(name="m2ps", bufs=3, space="PSUM") as m2ps,
                tc.tile_pool(name="xs", bufs=2) as xsp,
                tc.tile_pool(name="xsT", bufs=2) as xtp,
                tc.tile_pool(name="xsd", bufs=2, space="DRAM") as xdp,
            ):
                prev = None
                for c in range(NCH):
                    xa = xpool.tile([VA, F1 * SC], BF, tag="xa")
                    xb = xpool.tile([VB, F1 * SC], BF, tag="xb")
                    nc.sync.dma_start(xa[:], xk[0:VA, c, :, :])
                    nc.sync.dma_start(xb[:], xk[VA:V, c, :, :])
                    xsd = xdp.tile([(K - 1) * F1, V * SC], BF, tag="xsd1")
                    conv_spans(c, (xa, xb), F1, xsp, xsd, m1ps, 0)
                    xsT = xtp.tile([K * F1, V * SC], BF, tag="xsT")
                    nc.sync.dma_start(xsT[0:F1, :], xkt[c, :, :])
                    nc.sync.dma_start(xsT[F1:K * F1, :], xsd[:, :])
                    if prev is not None:
                        proj_evac(prev[0], stscr1, m2ps, mm_l1(prev[1]))
                    prev = (c, xsT)
                proj_evac(prev[0], stscr1, m2ps, mm_l1(prev[1]))
            bn_finalize(stscr1, gb1t, par1, "1")

            # ---- layer 2 ----
            with (
                tc.tile_pool(name="h1", bufs=2) as h1p,
                tc.tile_pool(name="m1ps2", bufs=4, space="PSUM") as m1ps2,
                tc.tile_pool(name="m2ps2", bufs=3, space="PSUM") as m2ps2,
                tc.tile_pool(name="xs2", bufs=2) as xsp2,
                tc.tile_pool(name="xsT2", bufs=1) as xtp2,
                tc.tile_pool(name="xsd2", bufs=2, space="DRAM") as xdp2,
                tc.tile_pool(name="h1d", bufs=2, space="DRAM") as hdp,
            ):
                prev = None
                for c in range(NCH):
                    ysl = ysl_c(c)
                    # JIT normalize+relu of chunk c (all bands), in place
                    nc.scalar.activation(
                        yslab[:, c * CW:(c + 1) * CW],
                        yslab[:, c * CW:(c + 1) * CW],
                        mybir.ActivationFunctionType.Relu,
                        bias=par1[:, 1:2], scale=par1[:, 0:1])
                    h1d = hdp.tile([V, F2 * SC], BF, tag="h1d")
                    for j in range(4):
                        bw = UB[j + 1] - UB[j]
                        nc.scalar.dma_start(
                            h1d[UB[j]:UB[j + 1], :].rearrange(
                                "u (o s) -> o u s", o=F2, s=SC),
                            ysl[32 * j:32 * j + 32, 0:bw, :])
                    ha = h1p.tile([VA, F2 * SC], BF, tag="ha")
                    hb = h1p.tile([VB, F2 * SC], BF, tag="hb")
                    nc.sync.dma_start(ha[:], h1d[0:VA, :])
                    nc.sync.dma_start(hb[:], h1d[VA:V, :])
                    xsd2 = xdp2.tile([(K - 1) * F2, V * SC], BF, tag="xsd2")
                    conv_spans(c, (ha, hb), F2, xsp2, xsd2, m1ps2, 1)
                    xsT2 = xtp2.tile([(K - 1) * F2, V * SC], BF, tag="xsT2")
                    nc.sync.dma_start(xsT2[:, :], xsd2[:, :])
                    if prev is not None:
                        proj_evac(prev[0], stscr2, m2ps2, mm_l2(prev[1]))
                    prev = (c, xsT2)
                proj_evac(prev[0], stscr2, m2ps2, mm_l2(prev[1]))
            bn_finalize(stscr2, gb2t, par2, "2")

            # ---- final normalize + relu + store ----
            with tc.tile_pool(name="stg", bufs=2) as stg:
                for c in range(NCH):
                    sl = c * SC
                    so = stg.tile([128, CW], F32, tag="stg")
                    so3 = so[:, :].rearrange("p (u s) -> p u s", u=42, s=SC)
                    nc.scalar.activation(
                        so[:, :], yslab[:, c * CW:(c + 1) * CW],
                        mybir.ActivationFunctionType.Relu,
                        bias=par2[:, 1:2], scale=par2[:, 0:1])
                    for j in range(4):
                        u0, u1 = UB[j], UB[j + 1]
                        nc.gpsimd.dma_start(
                            out[:, u0:u1, sl:sl + SC],
                            so3[32 * j:32 * j + 32, 0:u1 - u0, :])
    nc.compile()
    return nc


def _host_prep(x, lap, w1, w2, g1, be1, g2, be2):
    lap64 = np.asarray(lap).astype(np.float64)
    T = [np.eye(V), lap64]
    for _ in range(2, K):
        T.append(2.0 * lap64 @ T[-1] - T[-2])
    tsk = np.concatenate([T[k].T for k in range(1, K)], axis=1)  # [162, 648]
    w1r = np.asarray(w1).reshape(K * F1, F2)
    w2r = np.asarray(w2).reshape(K * F2, F2)
    w2k14 = w2r[F2:]                       # k=1..4 rows [128, 32]
    w2k0q = np.tile(w2r[0:F2], (4, 1))     # k=0 rows replicated per band
    gb1 = np.stack([np.tile(np.asarray(g1), 4), np.tile(np.asarray(be1), 4)],
                   axis=1)
    gb2 = np.stack([np.tile(np.asarray(g2), 4), np.tile(np.asarray(be2), 4)],
                   axis=1)
    # per-row weight: n_row / total; rows 32j+o weigh band j
    nrow = np.repeat(np.array(BW, np.float64) * S, 32)
    denom = (1.0 if os.environ.get("K_SKIP_CC", "0") == "1" else 8.0) * V * S
    wrow = (nrow / denom).astype(np.float32)[:, None]
    common = {
        "tsk": tsk.astype(BF16),
        "w1r": w1r.astype(BF16),
        "w2k14": w2k14.astype(BF16), "w2k0q": w2k0q.astype(BF16),
        "gb1": gb1.astype(np.float32), "gb2": gb2.astype(np.float32),
        "wrow": wrow,
    }
    in_maps = []
    xf = np.asarray(x).reshape(2, F1, V, 4096)
    for core in range(N_CORES):
        b, q = core // 4, core % 4
        xs = xf[b, :, :, q * S:(q + 1) * S]            # [16, 162, 1024]
        xkv = xs.transpose(1, 0, 2).reshape(V, F1, NCH, SC)
        xkv = xkv.transpose(0, 2, 1, 3)                # [162, 8, 16, 128]
        xktv = xs.reshape(F1, V, NCH, SC).transpose(2, 0, 1, 3)
        m = dict(common)
        m["xk"] = np.ascontiguousarray(xkv).astype(BF16)
        m["xkt"] = np.ascontiguousarray(xktv).reshape(
            NCH, F1, V * SC).astype(BF16)
        in_maps.append(m)
    return in_maps


_CACHE = {}


def _run(in_maps, trace=False):
    if "nc" not in _CACHE:
        _CACHE["nc"] = build_program()
    return run_bass_kernel_spmd(
        _CACHE["nc"], in_maps, core_ids=list(range(N_CORES)), trace=trace)


def kernel(x, lap, w1, b1, g1, be1, w2, b2, g2, be2, _trace=False):
    # conv biases b1/b2 cancel exactly inside BatchNorm -> ignored
    in_maps = _host_prep(x, lap, w1, w2, g1, be1, g2, be2)
    res = _run(in_maps, trace=_trace)
    _CACHE["last_results"] = res
    full = np.empty((2, F2, V, 4096), np.float32)
    for core in range(N_CORES):
        b, q = core // 4, core % 4
        full[b, :, :, q * S:(q + 1) * S] = res.results[core]["out"]
    return full.reshape(2, F2, V, 16, 16, 16)


# revision 9
# speedup vs baseline: 5.8295x; 1.0131x over previous
"""Trainium2 Bass kernel for nn_Block_39513699123558 (gnn_message_passing).

Two layers of (Chebyshev graph conv K=5 -> BatchNorm -> ReLU) on
x[B=2, F0=16, V=162, X=Y=Z=16].

Strategy (8 NeuronCores, data-parallel over B x S-quarters):
  - each core owns shard [Fin, V, S=1024] (b = core//4, s-quarter = core%4)
  - s is processed in NCH=8 chunks of SC=128 columns
  - Chebyshev k=1..4 via one host-precomputed T-stack (shared by both
    layers, T_k symmetric); T_0 = identity handled without matmuls
  - conv: matmul contracting V (lhsT = T-stack tiles, rhs = activations
    [v, (f s)]) -> xs[(k u), (f s)]
  - layout bridge xs -> xsT[(k f), (u s)] goes through small DRAM staging
    buffers: per-tile span writes (gpsimd software DGE, cheap descriptors)
    + one contiguous read back per chunk.  This keeps the dma_start count
    ~30x below the per-row SBUF bridge (each dma_start costs ~0.6us on
    the issuing sequencer + HWDGE).
  - L1 projection: single matmul contracting (k f)=80 (identity rows of
    xsT filled from a host-transposed copy of x read straight from HBM).
    Vertex axis split in 4 bands of 42/42/42/36 on PE column strips.
  - L2 projection: 2 accumulated matmuls: k=1..4 from xsT2 (128 rows)
    + k=0 read directly from the normalized y1 slab (rhs partitions
    32j..32j+32, replicated w2[0] weights, tile_position (32j, 32j)).
  - projection psum tiles cover r=3 vertex rows x SC per band strip;
    bn_stats off each psum tile, count-weighted AllReduce at layer end,
    per-partition scale/shift + ReLU applied lazily (JIT) next pass.
  - proj of chunk c is emitted after conv of chunk c+1 so the DRAM
    read-back latency hides under conv; xsT pools stay single/double
    buffered within SBUF limits.
All matmul data bf16; PSUM/stats/normalization math f32; output f32.
"""

import os
import sys

sys.path.insert(0, "/opt/trn_rl_repo")

SKIP_CC = os.environ.get("K_SKIP_CC", "0") == "1"
CC_MODE = os.environ.get("K_CC", "both")  # none|l1|l2|both


import numpy as np
import ml_dtypes

from concourse import bass, bacc, mybir
from concourse import tile
from concourse.bass_utils import run_bass_kernel_spmd

BF16 = ml_dtypes.bfloat16
BF = mybir.dt.bfloat16
F32 = mybir.dt.float32

V = 162
VA = 128
VB = V - VA  # 34
F1, F2 = 16, 32
K = 5
S = 1024          # s-columns per core
SC = 128          # s-chunk
NCH = S // SC     # 8
EPS = 1e-5
N_CORES = 8

# vertex bands per PE column strip: u in [UB[j], UB[j+1])
UB = [0, 42, 84, 126, 162]
BW = [42, 42, 42, 36]
R = 3             # u-rows per proj psum slot
NG = 14           # slots per band (band 3 uses 12 of 14)
NGJ = [14, 14, 14, 12]
CW = 42 * SC      # yslab cols per chunk

# T-stack k=1..4 (T_0 = I handled separately), 648 rows, 6 tiles of 108
ST = [108] * 6
SP = [(1, 0, 0, 0, 108), (1, 1, 0, 108, 54),
      (2, 1, 54, 0, 54), (2, 2, 0, 54, 108),
      (3, 3, 0, 0, 108), (3, 4, 0, 108, 54),
      (4, 4, 54, 0, 54), (4, 5, 0, 54, 108)]
SP_BY_T = [[sp for sp in SP if sp[1] == t] for t in range(6)]

NSUB = R * SC                 # proj psum cols (f32)
NSLOT = NCH * NG              # 112 bn-stats slots (chunk, g)
STSCR_W = NSLOT * 8


def build_program():
    nc = bacc.Bacc("TRN2", target_bir_lowering=False)
    xk = nc.declare_dram_parameter("xk", [V, NCH, F1, SC], BF, False)
    xkt = nc.declare_dram_parameter("xkt", [NCH, F1, V * SC], BF, False)
    tsk = nc.declare_dram_parameter("tsk", [V, 648], BF, False)
    w1r = nc.declare_dram_parameter("w1r", [K * F1, F2], BF, False)
    w2k14 = nc.declare_dram_parameter("w2k14", [128, F2], BF, False)
    w2k0q = nc.declare_dram_parameter("w2k0q", [128, F2], BF, False)
    gb1 = nc.declare_dram_parameter("gb1", [128, 2], F32, False)
    gb2 = nc.declare_dram_parameter("gb2", [128, 2], F32, False)
    wrow = nc.declare_dram_parameter("wrow", [128, 1], F32, False)
    out = nc.declare_dram_parameter("out", [F2, V, S], F32, isOutput=True)

    with tile.TileContext(nc) as tc:
        with (
            tc.tile_pool(name="consts", bufs=1) as cpool,
            tc.tile_pool(name="slab", bufs=1) as slab,
            tc.tile_pool(name="stats", bufs=1) as spool,
            tc.tile_pool(name="dram", bufs=1, space="DRAM") as dram,
        ):
            tA = cpool.tile([VA, 648], BF)
            tB = cpool.tile([VB, 648], BF)
            w1t = cpool.tile([K * F1, F2], BF)
            w2at = cpool.tile([128, F2], BF)
            w2bt = cpool.tile([128, F2], BF)
            gb1t = cpool.tile([128, 2], F32)
            gb2t = cpool.tile([128, 2], F32)
            wrt = cpool.tile([128, 1], F32)
            nc.sync.dma_start(tA[:], tsk[0:VA, :])
            nc.sync.dma_start(tB[:], tsk[VA:V, :])
            nc.sync.dma_start(w1t[:], w1r[:])
            nc.sync.dma_start(w2at[:], w2k14[:])
            nc.sync.dma_start(w2bt[:], w2k0q[:])
            nc.sync.dma_start(gb1t[:], gb1[:])
            nc.sync.dma_start(gb2t[:], gb2[:])
            nc.sync.dma_start(wrt[:], wrow[:])

            # y-slab: rows 32j+o; cols per chunk c: (u-local 42, s SC)
            yslab = slab.tile([128, NCH * CW], BF)

            def ysl_c(c):
                return yslab[:, c * CW:(c + 1) * CW].rearrange(
                    "p (u s) -> p u s", u=42, s=SC)

            stscr1 = spool.tile([128, STSCR_W], F32)
            stscr2 = spool.tile([128, STSCR_W], F32)
            par1 = spool.tile([128, 2], F32)
            par2 = spool.tile([128, 2], F32)
            nc.gpsimd.memset(stscr1[:], 0.0)
            nc.gpsimd.memset(stscr2[:], 0.0)

            def proj_evac(c, stscr, mxps, mm_fn):
                # projection for chunk c: band j on column strip j; the
                # bands share one [128, NSUB] psum tile per u-slot g.
                # mm_fn(ps2, j, col0, c) emits the matmul(s) for a strip.
                ysl = ysl_c(c)
                for g in range(NG):
                    rows = 128 if g < 12 else 96
                    nj = 4 if g < 12 else 3
                    ps2 = mxps.tile([128, NSUB], F32, tag="m2ps")
                    for j in range(nj):
                        col0 = (UB[j] + R * g) * SC
                        mm_fn(ps2, j, col0, c)
                    slot = c * NG + g
                    st = stscr[0:rows, slot * 8:slot * 8 + 6]
                    dst = ysl[0:rows, R * g:R * g + R, :]
                    src = ps2[0:rows, :].rearrange("p (r s) -> p r s",
                                                   r=R, s=SC)
                    if g % 2 == 0:
                        nc.vector.tensor_copy(dst, src)
                    else:
                        nc.scalar.copy(dst, src)
                    # stats off the evacuated bf16 copy: frees the psum
                    # tile as soon as the copy retires
                    flat = yslab[0:rows,
                                 c * CW + R * g * SC:c * CW + (R * g + R) * SC]
                    nc.vector.bn_stats(st, flat)

            def conv_spans(c, rhs_pair, nf, xsp, xsd, m1ps, eng_flip):
                # graph-conv matmuls (k=1..4) + psum evac + DRAM span
                # writes for chunk c.  rhs_pair = (xa, xb) tiles
                # [V-part, nf*SC]; writes xsd rows (k-1)*nf+f.
                nspl = nf * SC // 512
                for m in range(6):
                    xst = xsp.tile([108, nf * SC], BF, tag="xst")
                    for spl in range(nspl):
                        ps = m1ps.tile([108, 512], F32, tag="m1ps")
                        for kc, (tt, xx) in enumerate(
                                ((tA, rhs_pair[0]), (tB, rhs_pair[1]))):
                            nc.tensor.matmul(
                                ps[:], tt[:, m * 108:(m + 1) * 108],
                                xx[:, spl * 512:(spl + 1) * 512],
                                start=(kc == 0), stop=(kc == 1))
                        dst = xst[:, spl * 512:(spl + 1) * 512]
                        if (m + spl + eng_flip) % 2 == 0:
                            nc.vector.tensor_copy(dst, ps[:])
                        else:
                            nc.scalar.copy(dst, ps[:])
                    for (k, t, r0, u0, span) in SP_BY_T[m]:
                        nc.gpsimd.dma_start(
                            xsd[(k - 1) * nf:k * nf,
                                u0 * SC:(u0 + span) * SC].rearrange(
                                "f (u s) -> u f s", u=span, s=SC),
                            xst[r0:r0 + span, :].rearrange(
                                "u (f s) -> u f s", f=nf, s=SC))

            def mm_l1(xsT):
                def mm(ps2, j, col0, c):
                    nc.tensor.matmul(
                        ps2[32 * j:32 * j + 32, :], w1t[:],
                        xsT[:, col0:col0 + NSUB],
                        start=True, stop=True, tile_position=(0, 32 * j))
                return mm

            def mm_l2(xsT2):
                def mm(ps2, j, col0, c):
                    nc.tensor.matmul(
                        ps2[32 * j:32 * j + 32, :], w2at[:],
                        xsT2[:, col0:col0 + NSUB],
                        start=True, stop=False, tile_position=(0, 32 * j))
                    g = (col0 // SC - UB[j]) // R
                    nc.tensor.matmul(
                        ps2[32 * j:32 * j + 32, :],
                        w2bt[32 * j:32 * j + 32, :],
                        ysl_c(c)[32 * j:32 * j + 32,
                                 R * g:R * g + R, :],
                        start=False, stop=True,
                        tile_position=(32 * j, 32 * j))
                return mm

            def bn_finalize(stscr, gbt, par, tag):
                # per-row (mean, var) -> count-weighted (E, S) -> AllReduce
                # -> band-fold -> scale/shift
                sv = stscr[:, :].rearrange("p (n e) -> p n e", n=NSLOT, e=8)
                mv = spool.tile([128, 2], F32, tag=f"mv{tag}")
                nc.vector.bn_aggr(mv[:], sv[:, :, 0:6])
                es = spool.tile([128, 2], F32, tag=f"es{tag}")
                nc.vector.tensor_mul(es[:, 1:2], mv[:, 0:1], mv[:, 0:1])
                nc.vector.tensor_add(es[:, 1:2], es[:, 1:2], mv[:, 1:2])
                nc.vector.tensor_copy(es[:, 0:1], mv[:, 0:1])
                nc.vector.tensor_mul(es[:, 0:1], es[:, 0:1], wrt[:, 0:1])
                nc.vector.tensor_mul(es[:, 1:2], es[:, 1:2], wrt[:, 0:1])
                cin = dram.tile([128, 2], F32, tag=f"cin{tag}")
                cout = dram.tile([128, 2], F32, tag=f"cout{tag}")
                nc.gpsimd.dma_start(cin[:], es[:])
                use_cc = (CC_MODE == "both" or CC_MODE == ("l" + tag)) \
                    and not SKIP_CC
                if use_cc:
                    nc.gpsimd.collective_compute(
                        "AllReduce", mybir.AluOpType.add,
                        replica_groups=[list(range(N_CORES))],
                        ins=[cin[:].opt()], outs=[cout[:].opt()])
                else:
                    nc.gpsimd.dma_start(cout[:], cin[:])
                qs = spool.tile([32, 8], F32, tag=f"qs{tag}")
                nc.sync.dma_start(
                    qs[:].rearrange("o (j e) -> o j e", j=4, e=2),
                    cout[:].rearrange("(j o) e -> o j e", j=4, o=32))
                acc = spool.tile([32, 6], F32, tag=f"acc{tag}")
                nc.vector.tensor_add(acc[:, 0:2], qs[:, 0:2], qs[:, 2:4])
                nc.vector.tensor_add(acc[:, 2:4], qs[:, 4:6], qs[:, 6:8])
                nc.vector.tensor_add(acc[:, 0:2], acc[:, 0:2], acc[:, 2:4])
                # acc[:,0]=global mean, acc[:,1]=global E[y^2]
                nc.vector.tensor_mul(acc[:, 2:3], acc[:, 0:1], acc[:, 0:1])
                nc.vector.tensor_sub(acc[:, 1:2], acc[:, 1:2], acc[:, 2:3])
                nc.vector.tensor_scalar_add(acc[:, 1:2], acc[:, 1:2], EPS)
                nc.scalar.sqrt(acc[:, 2:3], acc[:, 1:2])
                nc.vector.reciprocal(acc[:, 3:4], acc[:, 2:3])
                nc.vector.tensor_mul(acc[:, 4:5], gbt[0:32, 0:1], acc[:, 3:4])
                nc.vector.tensor_mul(acc[:, 5:6], acc[:, 0:1], acc[:, 4:5])
                nc.vector.tensor_sub(acc[:, 5:6], gbt[0:32, 1:2], acc[:, 5:6])
                for j in range(4):
                    nc.sync.dma_start(par[32 * j:32 * j + 32, 0:1],
                                      acc[:, 4:5])
                    nc.sync.dma_start(par[32 * j:32 * j + 32, 1:2],
                                      acc[:, 5:6])

            # ---- layer 1 ----
            with (
                tc.tile_pool(name="x", bufs=2) as xpool,
                tc.tile_pool(name="m1ps", bufs=3, space="PSUM") as m1ps,
                tc.tile_pool(name="m2ps", bufs=5, space="PSUM") as m2ps,
                tc.tile_pool(name="xs", bufs=2) as xsp,
                tc.tile_pool(name="xsT", bufs=2) as xtp,
                tc.tile_pool(name="xsd", bufs=2, space="DRAM") as xdp,
            ):
                prev = None
                for c in range(NCH):
                    xa = xpool.tile([VA, F1 * SC], BF, tag="xa")
                    xb = xpool.tile([VB, F1 * SC], BF, tag="xb")
                    nc.sync.dma_start(xa[:], xk[0:VA, c, :, :])
                    nc.sync.dma_start(xb[:], xk[VA:V, c, :, :])
                    xsd = xdp.tile([(K - 1) * F1, V * SC], BF, tag="xsd1")
                    conv_spans(c, (xa, xb), F1, xsp, xsd, m1ps, 0)
                    xsT = xtp.tile([K * F1, V * SC], BF, tag="xsT")
                    nc.sync.dma_start(xsT[0:F1, :], xkt[c, :, :])
                    nc.sync.dma_start(xsT[F1:K * F1, :], xsd[:, :])
                    if prev is not None:
                        proj_evac(prev[0], stscr1, m2ps, mm_l1(prev[1]))
                    prev = (c, xsT)
                proj_evac(prev[0], stscr1, m2ps, mm_l1(prev[1]))
            bn_finalize(stscr1, gb1t, par1, "1")

            # ---- layer 2 ----
            with (
                tc.tile_pool(name="h1", bufs=2) as h1p,
                tc.tile_pool(name="m1ps2", bufs=3, space="PSUM") as m1ps2,
                tc.tile_pool(name="m2ps2", bufs=5, space="PSUM") as m2ps2,
                tc.tile_pool(name="xs2", bufs=3) as xsp2,
                tc.tile_pool(name="xsT2", bufs=1) as xtp2,
                tc.tile_pool(name="xsd2", bufs=2, space="DRAM") as xdp2,
                tc.tile_pool(name="h1d", bufs=2, space="DRAM") as hdp,
            ):
                prev = None
                for c in range(NCH):
                    ysl = ysl_c(c)
                    # JIT normalize+relu of chunk c (all bands), in place
                    nc.scalar.activation(
                        yslab[:, c * CW:(c + 1) * CW],
                        yslab[:, c * CW:(c + 1) * CW],
                        mybir.ActivationFunctionType.Relu,
                        bias=par1[:, 1:2], scale=par1[:, 0:1])
                    h1d = hdp.tile([V, F2 * SC], BF, tag="h1d")
                    for j in range(4):
                        bw = UB[j + 1] - UB[j]
                        nc.scalar.dma_start(
                            h1d[UB[j]:UB[j + 1], :].rearrange(
                                "u (o s) -> o u s", o=F2, s=SC),
                            ysl[32 * j:32 * j + 32, 0:bw, :])
                    ha = h1p.tile([VA, F2 * SC], BF, tag="ha")
                    hb = h1p.tile([VB, F2 * SC], BF, tag="hb")
                    nc.sync.dma_start(ha[:], h1d[0:VA, :])
                    nc.sync.dma_start(hb[:], h1d[VA:V, :])
                    xsd2 = xdp2.tile([(K - 1) * F2, V * SC], BF, tag="xsd2")
                    conv_spans(c, (ha, hb), F2, xsp2, xsd2, m1ps2, 1)
                    xsT2 = xtp2.tile([(K - 1) * F2, V * SC], BF, tag="xsT2")
                    nc.sync.dma_start(xsT2[:, :], xsd2[:, :])
                    if prev is not None:
                        proj_evac(prev[0], stscr2, m2ps2, mm_l2(prev[1]))
                    prev = (c, xsT2)
                proj_evac(prev[0], stscr2, m2ps2, mm_l2(prev[1]))
            bn_finalize(stscr2, gb2t, par2, "2")

            # ---- final normalize + relu + store ----
            with tc.tile_pool(name="stg", bufs=2) as stg:
                for c in range(NCH):
                    sl = c * SC
                    so = stg.tile([128, CW], F32, tag="stg")
                    so3 = so[:, :].rearrange("p (u s) -> p u s", u=42, s=SC)
                    nc.scalar.activation(
                        so[:, :], yslab[:, c * CW:(c + 1) * CW],
                        mybir.ActivationFunctionType.Relu,
                        bias=par2[:, 1:2], scale=par2[:, 0:1])
                    for j in range(4):
                        u0, u1 = UB[j], UB[j + 1]
                        nc.gpsimd.dma_start(
                            out[:, u0:u1, sl:sl + SC],
                            so3[32 * j:32 * j + 32, 0:u1 - u0, :])
    nc.compile()
    return nc


def _host_prep(x, lap, w1, w2, g1, be1, g2, be2):
    lap64 = np.asarray(lap).astype(np.float64)
    T = [np.eye(V), lap64]
    for _ in range(2, K):
        T.append(2.0 * lap64 @ T[-1] - T[-2])
    tsk = np.concatenate([T[k].T for k in range(1, K)], axis=1)  # [162, 648]
    w1r = np.asarray(w1).reshape(K * F1, F2)
    w2r = np.asarray(w2).reshape(K * F2, F2)
    w2k14 = w2r[F2:]                       # k=1..4 rows [128, 32]
    w2k0q = np.tile(w2r[0:F2], (4, 1))     # k=0 rows replicated per band
    gb1 = np.stack([np.tile(np.asarray(g1), 4), np.tile(np.asarray(be1), 4)],
                   axis=1)
    gb2 = np.stack([np.tile(np.asarray(g2), 4), np.tile(np.asarray(be2), 4)],
                   axis=1)
    # per-row weight: n_row / total; rows 32j+o weigh band j
    nrow = np.repeat(np.array(BW, np.float64) * S, 32)
    denom = (1.0 if os.environ.get("K_SKIP_CC", "0") == "1" else 8.0) * V * S
    wrow = (nrow / denom).astype(np.float32)[:, None]
    common = {
        "tsk": tsk.astype(BF16),
        "w1r": w1r.astype(BF16),
        "w2k14": w2k14.astype(BF16), "w2k0q": w2k0q.astype(BF16),
        "gb1": gb1.astype(np.float32), "gb2": gb2.astype(np.float32),
        "wrow": wrow,
    }
    in_maps = []
    xf = np.asarray(x).reshape(2, F1, V, 4096)
    for core in range(N_CORES):
        b, q = core // 4, core % 4
        xs = xf[b, :, :, q * S:(q + 1) * S]            # [16, 162, 1024]
        xkv = xs.transpose(1, 0, 2).reshape(V, F1, NCH, SC)
        xkv = xkv.transpose(0, 2, 1, 3)                # [162, 8, 16, 128]
        xktv = xs.reshape(F1, V, NCH, SC).transpose(2, 0, 1, 3)
        m = dict(common)
        m["xk"] = np.ascontiguousarray(xkv).astype(BF16)
        m["xkt"] = np.ascontiguousarray(xktv).reshape(
            NCH, F1, V * SC).astype(BF16)
        in_maps.append(m)
    return in_maps


_CACHE = {}


def _run(in_maps, trace=False):
    if "nc" not in _CACHE:
        _CACHE["nc"] = build_program()
    return run_bass_kernel_spmd(
        _CACHE["nc"], in_maps, core_ids=list(range(N_CORES)), trace=trace)


def kernel(x, lap, w1, b1, g1, be1, w2, b2, g2, be2, _trace=False):
    # conv biases b1/b2 cancel exactly inside BatchNorm -> ignored
    in_maps = _host_prep(x, lap, w1, w2, g1, be1, g2, be2)
    res = _run(in_maps, trace=_trace)
    _CACHE["last_results"] = res
    full = np.empty((2, F2, V, 4096), np.float32)
    for core in range(N_CORES):
        b, q = core // 4, core % 4
        full[b, :, :, q * S:(q + 1) * S] = res.results[core]["out"]
    return full.reshape(2, F2, V, 16, 16, 16)


# revision 13
# speedup vs baseline: 5.8576x; 1.0048x over previous
"""Trainium2 Bass kernel for nn_Block_39513699123558 (gnn_message_passing).

Two layers of (Chebyshev graph conv K=5 -> BatchNorm -> ReLU) on
x[B=2, F0=16, V=162, X=Y=Z=16].

Strategy (8 NeuronCores, data-parallel over B x S-quarters):
  - each core owns shard [Fin, V, S=1024] (b = core//4, s-quarter = core%4)
  - s is processed in NCH=8 chunks of SC=128 columns
  - Chebyshev k=1..4 via one host-precomputed T-stack (shared by both
    layers, T_k symmetric); T_0 = identity handled without matmuls
  - conv: matmul contracting V (lhsT = T-stack tiles, rhs = activations
    [v, (f s)]) -> xs[(k u), (f s)]
  - layout bridge xs -> xsT[(k f), (u s)] goes through small DRAM staging
    buffers: per-tile span writes (gpsimd software DGE, cheap descriptors)
    + one contiguous read back per chunk.  This keeps the dma_start count
    ~30x below the per-row SBUF bridge (each dma_start costs ~0.6us on
    the issuing sequencer + HWDGE).
  - L1 projection: single matmul contracting (k f)=80 (identity rows of
    xsT filled from a host-transposed copy of x read straight from HBM).
    Vertex axis split in 4 bands of 42/42/42/36 on PE column strips.
  - L2 projection: 2 accumulated matmuls: k=1..4 from xsT2 (128 rows)
    + k=0 read directly from the normalized y1 slab (rhs partitions
    32j..32j+32, replicated w2[0] weights, tile_position (32j, 32j)).
  - projection psum tiles cover r=3 vertex rows x SC per band strip;
    bn_stats off each psum tile, count-weighted AllReduce at layer end,
    per-partition scale/shift + ReLU applied lazily (JIT) next pass.
  - proj of chunk c is emitted after conv of chunk c+1 so the DRAM
    read-back latency hides under conv; xsT pools stay single/double
    buffered within SBUF limits.
All matmul data bf16; PSUM/stats/normalization math f32; output f32.
"""

import os
import sys

sys.path.insert(0, "/opt/trn_rl_repo")

SKIP_CC = os.environ.get("K_SKIP_CC", "0") == "1"
CC_MODE = os.environ.get("K_CC", "both")  # none|l1|l2|both


import numpy as np
import ml_dtypes

from concourse import bass, bacc, mybir
from concourse import tile
from concourse.bass_utils import run_bass_kernel_spmd

BF16 = ml_dtypes.bfloat16
BF = mybir.dt.bfloat16
F32 = mybir.dt.float32

V = 162
VA = 128
VB = V - VA  # 34
F1, F2 = 16, 32
K = 5
S = 1024          # s-columns per core
SC = 128          # s-chunk
NCH = S // SC     # 8
EPS = 1e-5
N_CORES = 8

# vertex bands per PE column strip: u in [UB[j], UB[j+1])
UB = [0, 42, 84, 126, 162]
BW = [42, 42, 42, 36]
R = 3             # u-rows per proj psum slot
NG = 14           # slots per band (band 3 uses 12 of 14)
NGJ = [14, 14, 14, 12]
CW = 42 * SC      # yslab cols per chunk

# T-stack k=1..4 (T_0 = I handled separately), 648 rows, 6 tiles of 108
ST = [108] * 6
SP = [(1, 0, 0, 0, 108), (1, 1, 0, 108, 54),
      (2, 1, 54, 0, 54), (2, 2, 0, 54, 108),
      (3, 3, 0, 0, 108), (3, 4, 0, 108, 54),
      (4, 4, 54, 0, 54), (4, 5, 0, 54, 108)]
SP_BY_T = [[sp for sp in SP if sp[1] == t] for t in range(6)]

NSUB = R * SC                 # proj psum cols (f32)
NSLOT = NCH * NG              # 112 bn-stats slots (chunk, g)
STSCR_W = NSLOT * 8


def build_program():
    nc = bacc.Bacc("TRN2", target_bir_lowering=False)
    xk = nc.declare_dram_parameter("xk", [V, NCH, F1, SC], BF, False)
    xkt = nc.declare_dram_parameter("xkt", [NCH, F1, V * SC], BF, False)
    tsk = nc.declare_dram_parameter("tsk", [V, 648], BF, False)
    w1r = nc.declare_dram_parameter("w1r", [K * F1, F2], BF, False)
    w2k14 = nc.declare_dram_parameter("w2k14", [128, F2], BF, False)
    w2k0q = nc.declare_dram_parameter("w2k0q", [128, F2], BF, False)
    gb1 = nc.declare_dram_parameter("gb1", [128, 2], F32, False)
    gb2 = nc.declare_dram_parameter("gb2", [128, 2], F32, False)
    wrow = nc.declare_dram_parameter("wrow", [128, 1], F32, False)
    out = nc.declare_dram_parameter("out", [F2, V, S], F32, isOutput=True)

    with tile.TileContext(nc) as tc:
        with (
            tc.tile_pool(name="consts", bufs=1) as cpool,
            tc.tile_pool(name="slab", bufs=1) as slab,
            tc.tile_pool(name="stats", bufs=1) as spool,
            tc.tile_pool(name="dram", bufs=1, space="DRAM") as dram,
        ):
            tA = cpool.tile([VA, 648], BF)
            tB = cpool.tile([VB, 648], BF)
            w1t = cpool.tile([K * F1, F2], BF)
            w2at = cpool.tile([128, F2], BF)
            w2bt = cpool.tile([128, F2], BF)
            gb1t = cpool.tile([128, 2], F32)
            gb2t = cpool.tile([128, 2], F32)
            wrt = cpool.tile([128, 1], F32)
            nc.sync.dma_start(tA[:], tsk[0:VA, :])
            nc.sync.dma_start(tB[:], tsk[VA:V, :])
            nc.sync.dma_start(w1t[:], w1r[:])
            nc.sync.dma_start(w2at[:], w2k14[:])
            nc.sync.dma_start(w2bt[:], w2k0q[:])
            nc.sync.dma_start(gb1t[:], gb1[:])
            nc.sync.dma_start(gb2t[:], gb2[:])
            nc.sync.dma_start(wrt[:], wrow[:])

            # y-slab: rows 32j+o; cols per chunk c: (u-local 42, s SC)
            yslab = slab.tile([128, NCH * CW], BF)

            def ysl_c(c):
                return yslab[:, c * CW:(c + 1) * CW].rearrange(
                    "p (u s) -> p u s", u=42, s=SC)

            stscr1 = spool.tile([128, STSCR_W], F32)
            stscr2 = spool.tile([128, STSCR_W], F32)
            par1 = spool.tile([128, 2], F32)
            par2 = spool.tile([128, 2], F32)
            nc.gpsimd.memset(stscr1[:], 0.0)
            nc.gpsimd.memset(stscr2[:], 0.0)

            def proj_evac(c, stscr, mxps, mm_fn):
                # projection for chunk c: band j on column strip j; the
                # bands share one [128, NSUB] psum tile per u-slot g.
                # mm_fn(ps2, j, col0, c) emits the matmul(s) for a strip.
                ysl = ysl_c(c)
                for g in range(NG):
                    rows = 128 if g < 12 else 96
                    nj = 4 if g < 12 else 3
                    ps2 = mxps.tile([128, NSUB], F32, tag="m2ps")
                    for j in range(nj):
                        col0 = (UB[j] + R * g) * SC
                        mm_fn(ps2, j, col0, c)
                    slot = c * NG + g
                    st = stscr[0:rows, slot * 8:slot * 8 + 6]
                    dst = ysl[0:rows, R * g:R * g + R, :]
                    src = ps2[0:rows, :].rearrange("p (r s) -> p r s",
                                                   r=R, s=SC)
                    if g % 2 == 0:
                        nc.vector.tensor_copy(dst, src)
                    else:
                        nc.scalar.copy(dst, src)
                    # stats off the evacuated bf16 copy: frees the psum
                    # tile as soon as the copy retires
                    flat = yslab[0:rows,
                                 c * CW + R * g * SC:c * CW + (R * g + R) * SC]
                    nc.vector.bn_stats(st, flat)

            def conv_spans(c, rhs_pair, nf, xsp, xsd, m1ps, eng_flip):
                # graph-conv matmuls (k=1..4) + psum evac + DRAM span
                # writes for chunk c.  rhs_pair = (xa, xb) tiles
                # [V-part, nf*SC]; writes xsd rows (k-1)*nf+f.
                nspl = nf * SC // 512
                for m in range(6):
                    xst = xsp.tile([108, nf * SC], BF, tag="xst")
                    for spl in range(nspl):
                        ps = m1ps.tile([108, 512], F32, tag="m1ps")
                        for kc, (tt, xx) in enumerate(
                                ((tA, rhs_pair[0]), (tB, rhs_pair[1]))):
                            nc.tensor.matmul(
                                ps[:], tt[:, m * 108:(m + 1) * 108],
                                xx[:, spl * 512:(spl + 1) * 512],
                                start=(kc == 0), stop=(kc == 1))
                        dst = xst[:, spl * 512:(spl + 1) * 512]
                        if (m + spl + eng_flip) % 2 == 0:
                            nc.vector.tensor_copy(dst, ps[:])
                        else:
                            nc.scalar.copy(dst, ps[:])
                    for (k, t, r0, u0, span) in SP_BY_T[m]:
                        nc.gpsimd.dma_start(
                            xsd[(k - 1) * nf:k * nf,
                                u0 * SC:(u0 + span) * SC].rearrange(
                                "f (u s) -> u f s", u=span, s=SC),
                            xst[r0:r0 + span, :].rearrange(
                                "u (f s) -> u f s", f=nf, s=SC))

            def mm_l1(xsT):
                def mm(ps2, j, col0, c):
                    nc.tensor.matmul(
                        ps2[32 * j:32 * j + 32, :], w1t[:],
                        xsT[:, col0:col0 + NSUB],
                        start=True, stop=True, tile_position=(0, 32 * j))
                return mm

            def mm_l2(xsT2):
                def mm(ps2, j, col0, c):
                    nc.tensor.matmul(
                        ps2[32 * j:32 * j + 32, :], w2at[:],
                        xsT2[:, col0:col0 + NSUB],
                        start=True, stop=False, tile_position=(0, 32 * j))
                    g = (col0 // SC - UB[j]) // R
                    nc.tensor.matmul(
                        ps2[32 * j:32 * j + 32, :],
                        w2bt[32 * j:32 * j + 32, :],
                        ysl_c(c)[32 * j:32 * j + 32,
                                 R * g:R * g + R, :],
                        start=False, stop=True,
                        tile_position=(32 * j, 32 * j))
                return mm

            def bn_finalize(stscr, gbt, par, tag):
                # per-row (mean, var) -> count-weighted (E, S) -> AllReduce
                # -> band-fold -> scale/shift
                sv = stscr[:, :].rearrange("p (n e) -> p n e", n=NSLOT, e=8)
                mv = spool.tile([128, 2], F32, tag=f"mv{tag}")
                nc.vector.bn_aggr(mv[:], sv[:, :, 0:6])
                es = spool.tile([128, 2], F32, tag=f"es{tag}")
                nc.vector.tensor_mul(es[:, 1:2], mv[:, 0:1], mv[:, 0:1])
                nc.vector.tensor_add(es[:, 1:2], es[:, 1:2], mv[:, 1:2])
                nc.vector.tensor_copy(es[:, 0:1], mv[:, 0:1])
                nc.vector.tensor_mul(es[:, 0:1], es[:, 0:1], wrt[:, 0:1])
                nc.vector.tensor_mul(es[:, 1:2], es[:, 1:2], wrt[:, 0:1])
                cin = dram.tile([128, 2], F32, tag=f"cin{tag}")
                cout = dram.tile([128, 2], F32, tag=f"cout{tag}")
                nc.gpsimd.dma_start(cin[:], es[:])
                use_cc = (CC_MODE == "both" or CC_MODE == ("l" + tag)) \
                    and not SKIP_CC
                if use_cc:
                    nc.gpsimd.collective_compute(
                        "AllReduce", mybir.AluOpType.add,
                        replica_groups=[list(range(N_CORES))],
                        ins=[cin[:].opt()], outs=[cout[:].opt()])
                else:
                    nc.gpsimd.dma_start(cout[:], cin[:])
                qs = spool.tile([32, 8], F32, tag=f"qs{tag}")
                nc.sync.dma_start(
                    qs[:].rearrange("o (j e) -> o j e", j=4, e=2),
                    cout[:].rearrange("(j o) e -> o j e", j=4, o=32))
                acc = spool.tile([32, 6], F32, tag=f"acc{tag}")
                nc.vector.tensor_add(acc[:, 0:2], qs[:, 0:2], qs[:, 2:4])
                nc.vector.tensor_add(acc[:, 2:4], qs[:, 4:6], qs[:, 6:8])
                nc.vector.tensor_add(acc[:, 0:2], acc[:, 0:2], acc[:, 2:4])
                # acc[:,0]=global mean, acc[:,1]=global E[y^2]
                nc.vector.tensor_mul(acc[:, 2:3], acc[:, 0:1], acc[:, 0:1])
                nc.vector.tensor_sub(acc[:, 1:2], acc[:, 1:2], acc[:, 2:3])
                nc.vector.tensor_scalar_add(acc[:, 1:2], acc[:, 1:2], EPS)
                nc.scalar.sqrt(acc[:, 2:3], acc[:, 1:2])
                nc.vector.reciprocal(acc[:, 3:4], acc[:, 2:3])
                nc.vector.tensor_mul(acc[:, 4:5], gbt[0:32, 0:1], acc[:, 3:4])
                nc.vector.tensor_mul(acc[:, 5:6], acc[:, 0:1], acc[:, 4:5])
                nc.vector.tensor_sub(acc[:, 5:6], gbt[0:32, 1:2], acc[:, 5:6])
                for j in range(4):
                    nc.sync.dma_start(par[32 * j:32 * j + 32, 0:1],
                                      acc[:, 4:5])
                    nc.sync.dma_start(par[32 * j:32 * j + 32, 1:2],
                                      acc[:, 5:6])

            # ---- layer 1 ----
            with (
                tc.tile_pool(name="x", bufs=2) as xpool,
                tc.tile_pool(name="m1ps", bufs=3, space="PSUM") as m1ps,
                tc.tile_pool(name="m2ps", bufs=5, space="PSUM") as m2ps,
                tc.tile_pool(name="xs", bufs=2) as xsp,
                tc.tile_pool(name="xsT", bufs=2) as xtp,
                tc.tile_pool(name="xsd", bufs=2, space="DRAM") as xdp,
            ):
                def l1_head(c):
                    xa = xpool.tile([VA, F1 * SC], BF, tag="xa")
                    xb = xpool.tile([VB, F1 * SC], BF, tag="xb")
                    nc.sync.dma_start(xa[:], xk[0:VA, c, :, :])
                    nc.sync.dma_start(xb[:], xk[VA:V, c, :, :])
                    return xa, xb

                prev = None
                xcur = l1_head(0)
                for c in range(NCH):
                    xsd = xdp.tile([(K - 1) * F1, V * SC], BF, tag="xsd1")
                    conv_spans(c, xcur, F1, xsp, xsd, m1ps, 0)
                    if c + 1 < NCH:
                        xcur = l1_head(c + 1)
                    xsT = xtp.tile([K * F1, V * SC], BF, tag="xsT")
                    nc.sync.dma_start(xsT[0:F1, :], xkt[c, :, :])
                    nc.sync.dma_start(xsT[F1:K * F1, :], xsd[:, :])
                    if prev is not None:
                        proj_evac(prev[0], stscr1, m2ps, mm_l1(prev[1]))
                    prev = (c, xsT)
                proj_evac(prev[0], stscr1, m2ps, mm_l1(prev[1]))
            bn_finalize(stscr1, gb1t, par1, "1")

            # ---- layer 2 ----
            with (
                tc.tile_pool(name="h1", bufs=2) as h1p,
                tc.tile_pool(name="m1ps2", bufs=3, space="PSUM") as m1ps2,
                tc.tile_pool(name="m2ps2", bufs=5, space="PSUM") as m2ps2,
                tc.tile_pool(name="xs2", bufs=3) as xsp2,
                tc.tile_pool(name="xsT2", bufs=1) as xtp2,
                tc.tile_pool(name="xsd2", bufs=2, space="DRAM") as xdp2,
                tc.tile_pool(name="h1d", bufs=2, space="DRAM") as hdp,
            ):
                def l2_head(c):
                    # JIT normalize+relu of chunk c (all bands), in place,
                    # then bridge to DRAM [u, (o s)] and load conv rhs
                    ysl = ysl_c(c)
                    nc.scalar.activation(
                        yslab[:, c * CW:(c + 1) * CW],
                        yslab[:, c * CW:(c + 1) * CW],
                        mybir.ActivationFunctionType.Relu,
                        bias=par1[:, 1:2], scale=par1[:, 0:1])
                    h1d = hdp.tile([V, F2 * SC], BF, tag="h1d")
                    for j in range(4):
                        bw = UB[j + 1] - UB[j]
                        nc.scalar.dma_start(
                            h1d[UB[j]:UB[j + 1], :].rearrange(
                                "u (o s) -> o u s", o=F2, s=SC),
                            ysl[32 * j:32 * j + 32, 0:bw, :])
                    ha = h1p.tile([VA, F2 * SC], BF, tag="ha")
                    hb = h1p.tile([VB, F2 * SC], BF, tag="hb")
                    nc.sync.dma_start(ha[:], h1d[0:VA, :])
                    nc.sync.dma_start(hb[:], h1d[VA:V, :])
                    return ha, hb

                prev = None
                hcur = l2_head(0)
                for c in range(NCH):
                    xsd2 = xdp2.tile([(K - 1) * F2, V * SC], BF, tag="xsd2")
                    conv_spans(c, hcur, F2, xsp2, xsd2, m1ps2, 1)
                    # emit next chunk's head before the lagged projection so
                    # its conv never queues behind the xsT2 read-back
                    if c + 1 < NCH:
                        hcur = l2_head(c + 1)
                    xsT2 = xtp2.tile([(K - 1) * F2, V * SC], BF, tag="xsT2")
                    nc.sync.dma_start(xsT2[:, :], xsd2[:, :])
                    if prev is not None:
                        proj_evac(prev[0], stscr2, m2ps2, mm_l2(prev[1]))
                    prev = (c, xsT2)
                proj_evac(prev[0], stscr2, m2ps2, mm_l2(prev[1]))
            bn_finalize(stscr2, gb2t, par2, "2")

            # ---- final normalize + relu + store ----
            with tc.tile_pool(name="stg", bufs=2) as stg:
                for c in range(NCH):
                    sl = c * SC
                    so = stg.tile([128, CW], F32, tag="stg")
                    so3 = so[:, :].rearrange("p (u s) -> p u s", u=42, s=SC)
                    nc.scalar.activation(
                        so[:, :], yslab[:, c * CW:(c + 1) * CW],
                        mybir.ActivationFunctionType.Relu,
                        bias=par2[:, 1:2], scale=par2[:, 0:1])
                    for j in range(4):
                        u0, u1 = UB[j], UB[j + 1]
                        nc.gpsimd.dma_start(
                            out[:, u0:u1, sl:sl + SC],
                            so3[32 * j:32 * j + 32, 0:u1 - u0, :])
    nc.compile()
    return nc


def _host_prep(x, lap, w1, w2, g1, be1, g2, be2):
    lap64 = np.asarray(lap).astype(np.float64)
    T = [np.eye(V), lap64]
    for _ in range(2, K):
        T.append(2.0 * lap64 @ T[-1] - T[-2])
    tsk = np.concatenate([T[k].T for k in range(1, K)], axis=1)  # [162, 648]
    w1r = np.asarray(w1).reshape(K * F1, F2)
    w2r = np.asarray(w2).reshape(K * F2, F2)
    w2k14 = w2r[F2:]                       # k=1..4 rows [128, 32]
    w2k0q = np.tile(w2r[0:F2], (4, 1))     # k=0 rows replicated per band
    gb1 = np.stack([np.tile(np.asarray(g1), 4), np.tile(np.asarray(be1), 4)],
                   axis=1)
    gb2 = np.stack([np.tile(np.asarray(g2), 4), np.tile(np.asarray(be2), 4)],
                   axis=1)
    # per-row weight: n_row / total; rows 32j+o weigh band j
    nrow = np.repeat(np.array(BW, np.float64) * S, 32)
    denom = (1.0 if os.environ.get("K_SKIP_CC", "0") == "1" else 8.0) * V * S
    wrow = (nrow / denom).astype(np.float32)[:, None]
    common = {
        "tsk": tsk.astype(BF16),
        "w1r": w1r.astype(BF16),
        "w2k14": w2k14.astype(BF16), "w2k0q": w2k0q.astype(BF16),
        "gb1": gb1.astype(np.float32), "gb2": gb2.astype(np.float32),
        "wrow": wrow,
    }
    in_maps = []
    xf = np.asarray(x).reshape(2, F1, V, 4096)
    for core in range(N_CORES):
        b, q = core // 4, core % 4
        xs = xf[b, :, :, q * S:(q + 1) * S]            # [16, 162, 1024]
        xkv = xs.transpose(1, 0, 2).reshape(V, F1, NCH, SC)
        xkv = xkv.transpose(0, 2, 1, 3)                # [162, 8, 16, 128]
        xktv = xs.reshape(F1, V, NCH, SC).transpose(2, 0, 1, 3)
        m = dict(common)
        m["xk"] = np.ascontiguousarray(xkv).astype(BF16)
        m["xkt"] = np.ascontiguousarray(xktv).reshape(
            NCH, F1, V * SC).astype(BF16)
        in_maps.append(m)
    return in_maps


_CACHE = {}


def _run(in_maps, trace=False):
    if "nc" not in _CACHE:
        _CACHE["nc"] = build_program()
    return run_bass_kernel_spmd(
        _CACHE["nc"], in_maps, core_ids=list(range(N_CORES)), trace=trace)


def kernel(x, lap, w1, b1, g1, be1, w2, b2, g2, be2, _trace=False):
    # conv biases b1/b2 cancel exactly inside BatchNorm -> ignored
    in_maps = _host_prep(x, lap, w1, w2, g1, be1, g2, be2)
    res = _run(in_maps, trace=_trace)
    _CACHE["last_results"] = res
    full = np.empty((2, F2, V, 4096), np.float32)
    for core in range(N_CORES):
        b, q = core // 4, core % 4
        full[b, :, :, q * S:(q + 1) * S] = res.results[core]["out"]
    return full.reshape(2, F2, V, 16, 16, 16)


# revision 18
# speedup vs baseline: 7.6500x; 1.3060x over previous
"""Trainium2 Bass kernel for nn_Block_39513699123558 (gnn_message_passing).

Two layers of (Chebyshev graph conv K=5 -> BatchNorm -> ReLU) on
x[B=2, F0=16, V=162, X=Y=Z=16].

Strategy (8 NeuronCores, data-parallel over B x S-quarters):
  - each core owns shard [Fin, V, S=1024] (b = core//4, s-quarter = core%4)
  - s is processed in NCH=8 chunks of SC=128 columns
  - Chebyshev k=1..4 via one host-precomputed T-stack (shared by both
    layers, T_k symmetric); T_0 = identity handled without matmuls
  - conv: matmul contracting V (lhsT = T-stack tiles, rhs = activations
    [v, (f s)]) -> xs[(k u), (f s)]
  - layout bridge xs -> xsT[(k f), (u s)] goes through small DRAM staging
    buffers: per-tile span writes (gpsimd software DGE, cheap descriptors)
    + one contiguous read back per chunk.  This keeps the dma_start count
    ~30x below the per-row SBUF bridge (each dma_start costs ~0.6us on
    the issuing sequencer + HWDGE).
  - L1 projection: single matmul contracting (k f)=80 (identity rows of
    xsT filled from a host-transposed copy of x read straight from HBM).
    Vertex axis split in 4 bands of 42/42/42/36 on PE column strips.
  - L2 projection: 2 accumulated matmuls: k=1..4 from xsT2 (128 rows)
    + k=0 read directly from the normalized y1 slab (rhs partitions
    32j..32j+32, replicated w2[0] weights, tile_position (32j, 32j)).
  - projection psum tiles cover r=3 vertex rows x SC per band strip;
    bn_stats off each psum tile, count-weighted AllReduce at layer end,
    per-partition scale/shift + ReLU applied lazily (JIT) next pass.
  - proj of chunk c is emitted after conv of chunk c+1 so the DRAM
    read-back latency hides under conv; xsT pools stay single/double
    buffered within SBUF limits.
All matmul data bf16; PSUM/stats/normalization math f32; output f32.
"""

import os
import sys

sys.path.insert(0, "/opt/trn_rl_repo")

SKIP_CC = os.environ.get("K_SKIP_CC", "0") == "1"
CC_MODE = os.environ.get("K_CC", "both")  # none|l1|l2|both


import numpy as np
import ml_dtypes

from concourse import bass, bacc, mybir
from concourse import tile
from concourse.bass_utils import run_bass_kernel_spmd

BF16 = ml_dtypes.bfloat16
BF = mybir.dt.bfloat16
F32 = mybir.dt.float32

V = 162
VA = 128
VB = V - VA  # 34
F1, F2 = 16, 32
K = 5
S = 1024          # s-columns per core
SC = 128          # s-chunk
NCH = S // SC     # 8
EPS = 1e-5
N_CORES = 8

# vertex bands per PE column strip: u in [UB[j], UB[j+1])
UB = [0, 42, 84, 126, 162]
BW = [42, 42, 42, 36]
R = 3             # u-rows per proj psum slot
NG = 14           # slots per band (band 3 uses 12 of 14)
NGJ = [14, 14, 14, 12]
CW = 42 * SC      # yslab cols per chunk

# T-stack k=1..4 (T_0 = I handled separately), 648 rows, 6 tiles of 108
ST = [108] * 6
SP = [(1, 0, 0, 0, 108), (1, 1, 0, 108, 54),
      (2, 1, 54, 0, 54), (2, 2, 0, 54, 108),
      (3, 3, 0, 0, 108), (3, 4, 0, 108, 54),
      (4, 4, 54, 0, 54), (4, 5, 0, 54, 108)]
SP_BY_T = [[sp for sp in SP if sp[1] == t] for t in range(6)]

NSUB = R * SC                 # proj psum cols (f32)
NSLOT = NCH * NG              # 112 bn-stats slots (chunk, g)
STSCR_W = NSLOT * 8


def build_program():
    nc = bacc.Bacc("TRN2", target_bir_lowering=False)
    xk = nc.declare_dram_parameter("xk", [V, NCH, F1, SC], BF, False)
    xkt = nc.declare_dram_parameter("xkt", [NCH, F1, V * SC], BF, False)
    tsk = nc.declare_dram_parameter("tsk", [V, 648], BF, False)
    w1r = nc.declare_dram_parameter("w1r", [K * F1, F2], BF, False)
    w2k14 = nc.declare_dram_parameter("w2k14", [128, F2], BF, False)
    w2k0q = nc.declare_dram_parameter("w2k0q", [128, F2], BF, False)
    gb1 = nc.declare_dram_parameter("gb1", [128, 2], F32, False)
    gb2 = nc.declare_dram_parameter("gb2", [128, 2], F32, False)
    wrow = nc.declare_dram_parameter("wrow", [128, 1], F32, False)
    out = nc.declare_dram_parameter("out", [F2, V, S], F32, isOutput=True)

    with tile.TileContext(nc) as tc:
        with (
            tc.tile_pool(name="consts", bufs=1) as cpool,
            tc.tile_pool(name="slab", bufs=1) as slab,
            tc.tile_pool(name="stats", bufs=1) as spool,
            tc.tile_pool(name="dram", bufs=1, space="DRAM") as dram,
        ):
            tA = cpool.tile([VA, 648], BF)
            tB = cpool.tile([VB, 648], BF)
            w1t = cpool.tile([K * F1, F2], BF)
            w2at = cpool.tile([128, F2], BF)
            w2bt = cpool.tile([128, F2], BF)
            gb1t = cpool.tile([128, 2], F32)
            gb2t = cpool.tile([128, 2], F32)
            wrt = cpool.tile([128, 1], F32)
            nc.sync.dma_start(tA[:], tsk[0:VA, :])
            nc.sync.dma_start(tB[:], tsk[VA:V, :])
            nc.sync.dma_start(w1t[:], w1r[:])
            nc.sync.dma_start(w2at[:], w2k14[:])
            nc.sync.dma_start(w2bt[:], w2k0q[:])
            nc.sync.dma_start(gb1t[:], gb1[:])
            nc.sync.dma_start(gb2t[:], gb2[:])
            nc.sync.dma_start(wrt[:], wrow[:])

            # y-slab: rows 32j+o; cols per chunk c: (u-local 42, s SC)
            yslab = slab.tile([128, NCH * CW], BF)

            def ysl_c(c):
                return yslab[:, c * CW:(c + 1) * CW].rearrange(
                    "p (u s) -> p u s", u=42, s=SC)

            stscr1 = spool.tile([128, STSCR_W], F32)
            stscr2 = spool.tile([128, STSCR_W], F32)
            par1 = spool.tile([128, 2], F32)
            par2 = spool.tile([128, 2], F32)
            nc.gpsimd.memset(stscr1[:], 0.0)
            nc.gpsimd.memset(stscr2[:], 0.0)

            def proj_evac(c, stscr, mxps, mm_fn):
                # projection for chunk c: band j on column strip j; the
                # bands share one [128, NSUB] psum tile per u-slot g.
                # mm_fn(ps2, j, col0, c, wave) emits one matmul wave for a
                # strip; waves are emitted band-major so the 4 strips run
                # concurrently on the PE array.
                ysl = ysl_c(c)
                for g in range(NG):
                    rows = 128 if g < 12 else 96
                    nj = 4 if g < 12 else 3
                    ps2 = mxps.tile([128, NSUB], F32, tag="m2ps")
                    for wave in range(mm_fn.waves):
                        for j in range(nj):
                            col0 = (UB[j] + R * g) * SC
                            mm_fn(ps2, j, col0, c, wave)
                    dst = ysl[0:rows, R * g:R * g + R, :]
                    src = ps2[0:rows, :].rearrange("p (r s) -> p r s",
                                                   r=R, s=SC)
                    if g % 2 == 0:
                        nc.vector.tensor_copy(dst, src)
                    else:
                        nc.scalar.copy(dst, src)
                    # stats off the evacuated bf16 copy: frees the psum
                    # tile as soon as the copy retires (bn_stats free-dim
                    # limit is 512, so one call per g)
                    slot = c * NG + g
                    st = stscr[0:rows, slot * 8:slot * 8 + 6]
                    flat = yslab[0:rows,
                                 c * CW + R * g * SC:c * CW + (R * g + R) * SC]
                    nc.vector.bn_stats(st, flat)

            def conv_spans(c, rhs_pair, nf, xsp, xsd, m1ps, eng_flip):
                # graph-conv matmuls (k=1..4) + psum evac + DRAM span
                # writes for chunk c.  rhs_pair = (xa, xb) tiles
                # [V-part, nf*SC]; writes xsd rows (k-1)*nf+f.
                nspl = nf * SC // 512
                for m in range(6):
                    xst = xsp.tile([108, nf * SC], BF, tag="xst")
                    for spl in range(nspl):
                        ps = m1ps.tile([108, 512], F32, tag="m1ps")
                        for kc, (tt, xx) in enumerate(
                                ((tA, rhs_pair[0]), (tB, rhs_pair[1]))):
                            nc.tensor.matmul(
                                ps[:], tt[:, m * 108:(m + 1) * 108],
                                xx[:, spl * 512:(spl + 1) * 512],
                                start=(kc == 0), stop=(kc == 1))
                        dst = xst[:, spl * 512:(spl + 1) * 512]
                        if (m + spl + eng_flip) % 2 == 0:
                            nc.vector.tensor_copy(dst, ps[:])
                        else:
                            nc.scalar.copy(dst, ps[:])
                    for (k, t, r0, u0, span) in SP_BY_T[m]:
                        nc.gpsimd.dma_start(
                            xsd[(k - 1) * nf:k * nf,
                                u0 * SC:(u0 + span) * SC].rearrange(
                                "f (u s) -> u f s", u=span, s=SC),
                            xst[r0:r0 + span, :].rearrange(
                                "u (f s) -> u f s", f=nf, s=SC))

            def mm_l1(xsT):
                def mm(ps2, j, col0, c, wave):
                    nc.tensor.matmul(
                        ps2[32 * j:32 * j + 32, :], w1t[:],
                        xsT[:, col0:col0 + NSUB],
                        start=True, stop=True, tile_position=(0, 32 * j))
                mm.waves = 1
                return mm

            def mm_l2(xsT2):
                # wave 0: k=1..4 on the 4 column strips (concurrent);
                # wave 1: k=0 on 4 disjoint diagonal 32x32 tiles
                def mm(ps2, j, col0, c, wave):
                    if wave == 0:
                        nc.tensor.matmul(
                            ps2[32 * j:32 * j + 32, :], w2at[:],
                            xsT2[:, col0:col0 + NSUB],
                            start=True, stop=False,
                            tile_position=(0, 32 * j))
                    else:
                        g = (col0 // SC - UB[j]) // R
                        nc.tensor.matmul(
                            ps2[32 * j:32 * j + 32, :],
                            w2bt[32 * j:32 * j + 32, :],
                            ysl_c(c)[32 * j:32 * j + 32,
                                     R * g:R * g + R, :],
                            start=False, stop=True,
                            tile_position=(32 * j, 32 * j))
                mm.waves = 2
                return mm

            def bn_finalize(stscr, gbt, par, tag):
                # per-row (mean, var) -> count-weighted (E, S) -> AllReduce
                # -> band-fold -> scale/shift
                sv = stscr[:, :].rearrange("p (n e) -> p n e", n=NSLOT, e=8)
                mv = spool.tile([128, 2], F32, tag=f"mv{tag}")
                nc.vector.bn_aggr(mv[:], sv[:, :, 0:6])
                es = spool.tile([128, 2], F32, tag=f"es{tag}")
                nc.vector.tensor_mul(es[:, 1:2], mv[:, 0:1], mv[:, 0:1])
                nc.vector.tensor_add(es[:, 1:2], es[:, 1:2], mv[:, 1:2])
                nc.vector.tensor_copy(es[:, 0:1], mv[:, 0:1])
                nc.vector.tensor_mul(es[:, 0:1], es[:, 0:1], wrt[:, 0:1])
                nc.vector.tensor_mul(es[:, 1:2], es[:, 1:2], wrt[:, 0:1])
                cin = dram.tile([128, 2], F32, tag=f"cin{tag}")
                cout = dram.tile([128, 2], F32, tag=f"cout{tag}")
                nc.gpsimd.dma_start(cin[:], es[:])
                use_cc = (CC_MODE == "both" or CC_MODE == ("l" + tag)) \
                    and not SKIP_CC
                if use_cc:
                    nc.gpsimd.collective_compute(
                        "AllReduce", mybir.AluOpType.add,
                        replica_groups=[list(range(N_CORES))],
                        ins=[cin[:].opt()], outs=[cout[:].opt()])
                else:
                    nc.gpsimd.dma_start(cout[:], cin[:])
                qs = spool.tile([32, 8], F32, tag=f"qs{tag}")
                nc.sync.dma_start(
                    qs[:].rearrange("o (j e) -> o j e", j=4, e=2),
                    cout[:].rearrange("(j o) e -> o j e", j=4, o=32))
                acc = spool.tile([32, 6], F32, tag=f"acc{tag}")
                nc.vector.tensor_add(acc[:, 0:2], qs[:, 0:2], qs[:, 2:4])
                nc.vector.tensor_add(acc[:, 2:4], qs[:, 4:6], qs[:, 6:8])
                nc.vector.tensor_add(acc[:, 0:2], acc[:, 0:2], acc[:, 2:4])
                # acc[:,0]=global mean, acc[:,1]=global E[y^2]
                nc.vector.tensor_mul(acc[:, 2:3], acc[:, 0:1], acc[:, 0:1])
                nc.vector.tensor_sub(acc[:, 1:2], acc[:, 1:2], acc[:, 2:3])
                nc.vector.tensor_scalar_add(acc[:, 1:2], acc[:, 1:2], EPS)
                nc.scalar.sqrt(acc[:, 2:3], acc[:, 1:2])
                nc.vector.reciprocal(acc[:, 3:4], acc[:, 2:3])
                nc.vector.tensor_mul(acc[:, 4:5], gbt[0:32, 0:1], acc[:, 3:4])
                nc.vector.tensor_mul(acc[:, 5:6], acc[:, 0:1], acc[:, 4:5])
                nc.vector.tensor_sub(acc[:, 5:6], gbt[0:32, 1:2], acc[:, 5:6])
                for j in range(4):
                    nc.sync.dma_start(par[32 * j:32 * j + 32, 0:1],
                                      acc[:, 4:5])
                    nc.sync.dma_start(par[32 * j:32 * j + 32, 1:2],
                                      acc[:, 5:6])

            # ---- layer 1 ----
            with (
                tc.tile_pool(name="x", bufs=2) as xpool,
                tc.tile_pool(name="m1ps", bufs=3, space="PSUM") as m1ps,
                tc.tile_pool(name="m2ps", bufs=5, space="PSUM") as m2ps,
                tc.tile_pool(name="xs", bufs=3) as xsp,
                tc.tile_pool(name="xsT", bufs=2) as xtp,
                tc.tile_pool(name="xsd", bufs=2, space="DRAM") as xdp,
            ):
                def l1_head(c):
                    xa = xpool.tile([VA, F1 * SC], BF, tag="xa")
                    xb = xpool.tile([VB, F1 * SC], BF, tag="xb")
                    nc.sync.dma_start(xa[:], xk[0:VA, c, :, :])
                    nc.sync.dma_start(xb[:], xk[VA:V, c, :, :])
                    return xa, xb

                prev = None
                xcur = l1_head(0)
                for c in range(NCH):
                    xsd = xdp.tile([(K - 1) * F1, V * SC], BF, tag="xsd1")
                    conv_spans(c, xcur, F1, xsp, xsd, m1ps, 0)
                    if c + 1 < NCH:
                        xcur = l1_head(c + 1)
                    xsT = xtp.tile([K * F1, V * SC], BF, tag="xsT")
                    nc.sync.dma_start(xsT[0:F1, :], xkt[c, :, :])
                    nc.sync.dma_start(xsT[F1:K * F1, :], xsd[:, :])
                    if prev is not None:
                        proj_evac(prev[0], stscr1, m2ps, mm_l1(prev[1]))
                    prev = (c, xsT)
                proj_evac(prev[0], stscr1, m2ps, mm_l1(prev[1]))
            bn_finalize(stscr1, gb1t, par1, "1")

            # ---- layer 2 ----
            with (
                tc.tile_pool(name="h1", bufs=2) as h1p,
                tc.tile_pool(name="m1ps2", bufs=3, space="PSUM") as m1ps2,
                tc.tile_pool(name="m2ps2", bufs=5, space="PSUM") as m2ps2,
                tc.tile_pool(name="xs2", bufs=3) as xsp2,
                tc.tile_pool(name="xsT2", bufs=1) as xtp2,
                tc.tile_pool(name="xsd2", bufs=2, space="DRAM") as xdp2,
                tc.tile_pool(name="h1d", bufs=2, space="DRAM") as hdp,
            ):
                def l2_head(c):
                    # JIT normalize+relu of chunk c (all bands), in place,
                    # then bridge to DRAM [u, (o s)] and load conv rhs
                    ysl = ysl_c(c)
                    nc.scalar.activation(
                        yslab[:, c * CW:(c + 1) * CW],
                        yslab[:, c * CW:(c + 1) * CW],
                        mybir.ActivationFunctionType.Relu,
                        bias=par1[:, 1:2], scale=par1[:, 0:1])
                    h1d = hdp.tile([V, F2 * SC], BF, tag="h1d")
                    for j in range(4):
                        bw = UB[j + 1] - UB[j]
                        nc.scalar.dma_start(
                            h1d[UB[j]:UB[j + 1], :].rearrange(
                                "u (o s) -> o u s", o=F2, s=SC),
                            ysl[32 * j:32 * j + 32, 0:bw, :])
                    ha = h1p.tile([VA, F2 * SC], BF, tag="ha")
                    hb = h1p.tile([VB, F2 * SC], BF, tag="hb")
                    nc.sync.dma_start(ha[:], h1d[0:VA, :])
                    nc.sync.dma_start(hb[:], h1d[VA:V, :])
                    return ha, hb

                prev = None
                hcur = l2_head(0)
                for c in range(NCH):
                    xsd2 = xdp2.tile([(K - 1) * F2, V * SC], BF, tag="xsd2")
                    conv_spans(c, hcur, F2, xsp2, xsd2, m1ps2, 1)
                    # emit next chunk's head before the lagged projection so
                    # its conv never queues behind the xsT2 read-back
                    if c + 1 < NCH:
                        hcur = l2_head(c + 1)
                    xsT2 = xtp2.tile([(K - 1) * F2, V * SC], BF, tag="xsT2")
                    nc.sync.dma_start(xsT2[:, :], xsd2[:, :])
                    if prev is not None:
                        proj_evac(prev[0], stscr2, m2ps2, mm_l2(prev[1]))
                    prev = (c, xsT2)
                proj_evac(prev[0], stscr2, m2ps2, mm_l2(prev[1]))
            bn_finalize(stscr2, gb2t, par2, "2")

            # ---- final normalize + relu + store ----
            with tc.tile_pool(name="stg", bufs=2) as stg:
                for c in range(NCH):
                    sl = c * SC
                    so = stg.tile([128, CW], F32, tag="stg")
                    so3 = so[:, :].rearrange("p (u s) -> p u s", u=42, s=SC)
                    nc.scalar.activation(
                        so[:, :], yslab[:, c * CW:(c + 1) * CW],
                        mybir.ActivationFunctionType.Relu,
                        bias=par2[:, 1:2], scale=par2[:, 0:1])
                    for j in range(4):
                        u0, u1 = UB[j], UB[j + 1]
                        nc.gpsimd.dma_start(
                            out[:, u0:u1, sl:sl + SC],
                            so3[32 * j:32 * j + 32, 0:u1 - u0, :])
    nc.compile()
    return nc


def _host_prep(x, lap, w1, w2, g1, be1, g2, be2):
    lap64 = np.asarray(lap).astype(np.float64)
    T = [np.eye(V), lap64]
    for _ in range(2, K):
        T.append(2.0 * lap64 @ T[-1] - T[-2])
    tsk = np.concatenate([T[k].T for k in range(1, K)], axis=1)  # [162, 648]
    w1r = np.asarray(w1).reshape(K * F1, F2)
    w2r = np.asarray(w2).reshape(K * F2, F2)
    w2k14 = w2r[F2:]                       # k=1..4 rows [128, 32]
    w2k0q = np.tile(w2r[0:F2], (4, 1))     # k=0 rows replicated per band
    gb1 = np.stack([np.tile(np.asarray(g1), 4), np.tile(np.asarray(be1), 4)],
                   axis=1)
    gb2 = np.stack([np.tile(np.asarray(g2), 4), np.tile(np.asarray(be2), 4)],
                   axis=1)
    # per-row weight: n_row / total; rows 32j+o weigh band j
    nrow = np.repeat(np.array(BW, np.float64) * S, 32)
    denom = (1.0 if os.environ.get("K_SKIP_CC", "0") == "1" else 8.0) * V * S
    wrow = (nrow / denom).astype(np.float32)[:, None]
    common = {
        "tsk": tsk.astype(BF16),
        "w1r": w1r.astype(BF16),
        "w2k14": w2k14.astype(BF16), "w2k0q": w2k0q.astype(BF16),
        "gb1": gb1.astype(np.float32), "gb2": gb2.astype(np.float32),
        "wrow": wrow,
    }
    in_maps = []
    xf = np.asarray(x).reshape(2, F1, V, 4096)
    for core in range(N_CORES):
        b, q = core // 4, core % 4
        xs = xf[b, :, :, q * S:(q + 1) * S]            # [16, 162, 1024]
        xkv = xs.transpose(1, 0, 2).reshape(V, F1, NCH, SC)
        xkv = xkv.transpose(0, 2, 1, 3)                # [162, 8, 16, 128]
        xktv = xs.reshape(F1, V, NCH, SC).transpose(2, 0, 1, 3)
        m = dict(common)
        m["xk"] = np.ascontiguousarray(xkv).astype(BF16)
        m["xkt"] = np.ascontiguousarray(xktv).reshape(
            NCH, F1, V * SC).astype(BF16)
        in_maps.append(m)
    return in_maps


_CACHE = {}


def _run(in_maps, trace=False):
    if "nc" not in _CACHE:
        _CACHE["nc"] = build_program()
    return run_bass_kernel_spmd(
        _CACHE["nc"], in_maps, core_ids=list(range(N_CORES)), trace=trace)


def kernel(x, lap, w1, b1, g1, be1, w2, b2, g2, be2, _trace=False):
    # conv biases b1/b2 cancel exactly inside BatchNorm -> ignored
    in_maps = _host_prep(x, lap, w1, w2, g1, be1, g2, be2)
    res = _run(in_maps, trace=_trace)
    _CACHE["last_results"] = res
    full = np.empty((2, F2, V, 4096), np.float32)
    for core in range(N_CORES):
        b, q = core // 4, core % 4
        full[b, :, :, q * S:(q + 1) * S] = res.results[core]["out"]
    return full.reshape(2, F2, V, 16, 16, 16)
